# revision 1
# baseline (speedup 1.0000x reference)
"""DiSco weighted loss kernel for 8 trn2 NeuronCores.

Math: for symmetric a_ij=|x_i-x_j|, the weighted distance-correlation terms
collapse to  dcov = S_ab - 2*T1ab + g_a*g_b  with
  ar_i = sum_j w_j a_ij,  g_a = sum_i w_i ar_i,  T1ab = sum_i w_i ar_i br_i,
  S_ab = sum_ij w_i w_j a_ij b_ij,
and dvar_x = 2(q - m^2) - 2*T1aa + g_a^2 exactly (|.|^2 loses the abs).
Each core owns 512 i-rows (free axis) and scans all j (partition axis,
32 column-sets of its [128,32] f-major tiles); TensorE reduces over j via
bf16 matmuls accumulated in PSUM; the per-core scalar partials are summed
on the host (8x22 floats) to avoid a ~20us AllReduce latency floor.
"""

from contextlib import ExitStack

import numpy as np
import ml_dtypes

import concourse.bass as bass
from concourse import mybir
from concourse.bass_utils import run_bass_kernel_spmd

F32 = mybir.dt.float32
BF16 = mybir.dt.bfloat16
I32 = mybir.dt.int32
U16 = mybir.dt.uint16
AO = mybir.AluOpType
AF = mybir.ActivationFunctionType
AX = mybir.AxisListType

N, C, NCORES = 4096, 3, 8
M = N // NCORES  # 512 rows per core
NB = 32  # j-sets (columns of the [128,32] tiles)

DISCO_LAMBDA = 0.1
EPS_W = 1e-8
EPS_VAR = 1e-10


def _build_program():
    nc = bass.Bass()
    lg = nc.declare_dram_parameter("lg", [N, C], F32, isOutput=False)
    lgm = nc.declare_dram_parameter("lgm", [M, C], F32, isOutput=False)
    tg = nc.declare_dram_parameter("tg", [N], I32, isOutput=False)
    wf = nc.declare_dram_parameter("wf", [N], F32, isOutput=False)
    wm = nc.declare_dram_parameter("wm", [1, M], F32, isOutput=False)
    m1f = nc.declare_dram_parameter("m1f", [N], BF16, isOutput=False)
    m2f = nc.declare_dram_parameter("m2f", [N], BF16, isOutput=False)
    m1m = nc.declare_dram_parameter("m1m", [1, M], BF16, isOutput=False)
    m2m = nc.declare_dram_parameter("m2m", [1, M], BF16, isOutput=False)
    out = nc.declare_dram_parameter("out", [32], F32, isOutput=True)
    scr = nc.dram_tensor("scr", [1, M], BF16)

    es = ExitStack()
    def sb(name, shp, dt):
        return es.enter_context(nc.sbuf_tensor(name, shp, dt))

    def ps(name, shp):
        return es.enter_context(nc.psum_tensor(name, shp, F32))

    lgt = sb("lgt", [128, 96], F32)
    lgtm = sb("lgtm", [128, 12], F32)
    tgt = sb("tgt", [128, 32], I32)
    wt = sb("wt", [128, 32], F32)
    m1t = sb("m1t", [128, 32], BF16)
    m2t = sb("m2t", [128, 32], BF16)
    wrow = sb("wrow", [2, M], F32)
    mrow = sb("mrow", [2, M], BF16)
    y1row = sb("y1row", [128, M], BF16)
    y2row = sb("y2row", [128, M], BF16)
    xrow = sb("xrow", [128, M], BF16)

    e = sb("e", [128, 96], F32)
    den = sb("den", [128, 32], F32)
    rec = sb("rec", [128, 32], F32)
    sc = sb("sc", [128, 32], F32)
    scbf = sb("scbf", [128, 32], BF16)
    sc_r = sb("sc_r", [128, 32], F32)  # bf16-rounded scores back in f32
    em = sb("em", [128, 12], F32)
    denm = sb("denm", [128, 4], F32)
    recm = sb("recm", [128, 4], F32)
    scm = sb("scm", [128, 4], F32)
    scmbf = sb("scmbf", [128, 4], BF16)
    ny1 = sb("ny1", [128, 32], F32)
    ny2 = sb("ny2", [128, 32], F32)
    msk1 = sb("msk1", [128, 32], F32)
    msk2 = sb("msk2", [128, 32], F32)
    w1 = sb("w1", [128, 32], F32)
    w2 = sb("w2", [128, 32], F32)
    w12 = sb("w12", [128, 64], BF16)
    m1ff = sb("m1ff", [128, 32], F32)
    m2ff = sb("m2ff", [128, 32], F32)
    sq = sb("sq", [128, 32], F32)  # scratch squares/products
    pr = sb("pr", [128, 32], F32)
    tgtf = sb("tgtf", [128, 32], F32)
    sel = sb("sel", [128, 32], F32)
    lt = sb("lt", [128, 32], F32)
    lnden = sb("lnden", [128, 32], F32)
    ce = sb("ce", [128, 32], F32)
    G = sb("G", [128, 12], F32)
    Gsb = sb("Gsb", [1, 12], F32)
    ones_t = sb("ones_t", [128, 1], F32)

    # double-buffered loop tiles
    d_ = [sb(f"d{i}", [128, 2 * M], BF16) for i in range(3)]
    a_ = [sb(f"a{i}", [128, 2 * M], BF16) for i in range(3)]
    b1_ = [sb(f"b1{i}", [128, 2 * M], BF16) for i in range(3)]
    b2_ = [sb(f"b2{i}", [128, 2 * M], BF16) for i in range(3)]
    ab1_ = [sb(f"ab1{i}", [128, 2 * M], BF16) for i in range(3)]
    ab2_ = [sb(f"ab2{i}", [128, 2 * M], BF16) for i in range(3)]

    mrow_w = sb("mrow_w", [2, M], F32)
    w12row = sb("w12row", [2, M], F32)
    arsb = sb("arsb", [2, M], F32)
    V = sb("V", [2, 2 * M], F32)
    t_ = sb("t_", [2, M], F32)
    t2_ = sb("t2_", [2, M], F32)
    sc2 = sb("sc2", [2, 8], F32)
    st_b2 = sb("st_b2", [2, M], F32)
    st_p2 = sb("st_p2", [2, M], F32)
    w12row_bf = sb("w12row_bf", [2, M], BF16)

    ps_a = ps("ps_a", [2, M])
    ps_b1 = ps("ps_b1", [2, M])
    ps_b2 = ps("ps_b2", [2, M])
    ps_p1 = ps("ps_p1", [2, M])
    ps_p2 = ps("ps_p2", [2, M])
    ps_g = ps("ps_g", [1, 12])

    dm = es.enter_context(nc.semaphore("dm"))
    sa = es.enter_context(nc.semaphore("sa"))  # ACT progress
    svp = es.enter_context(nc.semaphore("svp"))  # DVE phase-0 progress
    sv = es.enter_context(nc.semaphore("sv"))  # DVE loop progress
    sp = es.enter_context(nc.semaphore("sp"))  # PE progress
    sa2 = es.enter_context(nc.semaphore("sa2"))  # ACT phase-2 copies
    block = es.enter_context(nc.Block())

    @block.sync
    def _(sync):
        # 10 input DMAs (dm: 16 each -> 160 when all in)
        sync.dma_start(out=lgt[:], in_=lg[:].rearrange("(p f) c -> p (f c)", f=32)).then_inc(dm, 16)
        sync.dma_start(out=lgtm[:], in_=lgm[:].rearrange("(p q) c -> p (q c)", q=4)).then_inc(dm, 16)
        sync.dma_start(out=tgt[:], in_=tg[:].rearrange("(p f) -> p f", f=32)).then_inc(dm, 16)
        sync.dma_start(out=wt[:], in_=wf[:].rearrange("(p f) -> p f", f=32)).then_inc(dm, 16)
        sync.dma_start(out=m1t[:], in_=m1f[:].rearrange("(p f) -> p f", f=32)).then_inc(dm, 16)
        sync.dma_start(out=m2t[:], in_=m2f[:].rearrange("(p f) -> p f", f=32)).then_inc(dm, 16)
        sync.dma_start(out=wrow[:], in_=wm[:].broadcast_to([2, M])).then_inc(dm, 16)
        sync.dma_start(out=mrow[0:1, :], in_=m1m[:]).then_inc(dm, 16)
        sync.dma_start(out=mrow[1:2, :], in_=m2m[:]).then_inc(dm, 16)
        sync.dma_start(out=y1row[:], in_=m1m[:].broadcast_to([128, M])).then_inc(dm, 16)
        sync.dma_start(out=y2row[:], in_=m2m[:].broadcast_to([128, M])).then_inc(dm, 16)
        # scores-mine roundtrip: wait for scmbf (svp>=1)
        sync.wait_ge(svp, 1)
        sync.dma_start(out=scr[:].rearrange("a b -> (a b)"), in_=scmbf[:]).then_inc(dm, 16)
        sync.wait_ge(dm, 16 * 12)
        sync.dma_start(out=xrow[:], in_=scr[:].broadcast_to([128, M])).then_inc(dm, 16)
        # phase-2 row moves (partition 1 via DMA)
        sync.wait_ge(sa2, 4)
        sync.dma_start(out=V[1:2, 0:M], in_=st_b2[1:2, :]).then_inc(dm, 16)
        sync.dma_start(out=V[1:2, M : 2 * M], in_=st_p2[1:2, :]).then_inc(dm, 16)
        # outputs
        sync.wait_ge(sv, 200)
        sync.dma_start(out=out[0:12], in_=Gsb[:]).then_inc(dm, 16)
        sync.dma_start(out=out[12:28], in_=sc2[:]).then_inc(dm, 16)

    @block.scalar
    def _(scalar):
        scalar.wait_ge(dm, 16 * 11)  # all initial loads in
        scalar.activation(e[:], lgt[:], AF.Exp).then_inc(sa, 1)
        scalar.activation(em[:], lgtm[:], AF.Exp).then_inc(sa, 1)
        scalar.wait_ge(svp, 2)  # den ready
        scalar.activation(lnden[:], den[:], AF.Ln).then_inc(sa, 1)
        # loop: b1/b2 per j-set, double buffered
        scalar.wait_ge(svp, 3)  # ny1, ny2 ready
        for K in range(NB // 2):
            s = K % 3
            if K >= 3:
                scalar.wait_ge(sp, K - 1)
            k0, k1 = 2 * K, 2 * K + 1
            scalar.activation(b1_[s][:, 0:M], y1row[:], AF.Abs, bias=ny1[:, k0 : k0 + 1]).then_inc(sa, 1)
            scalar.activation(b1_[s][:, M : 2 * M], y1row[:], AF.Abs, bias=ny1[:, k1 : k1 + 1]).then_inc(sa, 1)
            scalar.activation(b2_[s][:, 0:M], y2row[:], AF.Abs, bias=ny2[:, k0 : k0 + 1]).then_inc(sa, 1)
            scalar.activation(b2_[s][:, M : 2 * M], y2row[:], AF.Abs, bias=ny2[:, k1 : k1 + 1]).then_inc(sa, 1)
        # phase 2 copies out of PSUM
        scalar.wait_ge(sp, NB // 2 + 1)
        scalar.activation(V[0:1, 0:M], ps_b1[0:1, :], AF.Copy).then_inc(sa2, 1)
        scalar.activation(V[0:1, M : 2 * M], ps_p1[0:1, :], AF.Copy).then_inc(sa2, 1)
        scalar.activation(st_b2[:], ps_b2[:], AF.Copy).then_inc(sa2, 1)
        scalar.activation(st_p2[:], ps_p2[:], AF.Copy).then_inc(sa2, 1)

    @block.vector
    def _(vector):
        # ---- phase 0 ----
        vector.memset(ones_t[:], 1.0)
        vector.drain()
        vector.wait_ge(sa, 2)
        vector.tensor_reduce(denm[:], em[:].rearrange("p (f c) -> p f c", c=3), AX.X, AO.add)
        vector.drain()
        vector.reciprocal(recm[:], denm[:])
        vector.drain()
        vector.tensor_tensor(scm[:], em[:, 0:12:3], recm[:], AO.mult)
        vector.drain()
        vector.tensor_copy(scmbf[:], scm[:])
        vector.drain().then_inc(svp, 1)
        vector.tensor_reduce(den[:], e[:].rearrange("p (f c) -> p f c", c=3), AX.X, AO.add)
        vector.drain()
        vector.reciprocal(rec[:], den[:])
        vector.drain()
        vector.tensor_tensor(sc[:], e[:, 0:96:3], rec[:], AO.mult)
        vector.drain()
        vector.tensor_copy(scbf[:], sc[:])
        vector.drain()
        vector.tensor_copy(sc_r[:], scbf[:])
        vector.drain().then_inc(svp, 1)
        vector.wait_ge(dm, 16 * 11)  # all initial loads in
        vector.tensor_scalar(ny1[:], m1t[:], -1.0, None, AO.mult)
        vector.drain()
        vector.tensor_scalar(ny2[:], m2t[:], -1.0, None, AO.mult)
        vector.drain().then_inc(svp, 1)
        vector.tensor_scalar(msk1[:], m1t[:], 0.0, None, AO.is_gt)
        vector.drain()
        vector.tensor_scalar(msk2[:], m2t[:], 0.0, None, AO.is_gt)
        vector.drain()
        vector.tensor_tensor(w1[:], wt[:], msk1[:], AO.mult)
        vector.drain()
        vector.tensor_tensor(w2[:], wt[:], msk2[:], AO.mult)
        vector.drain()
        vector.tensor_copy(w12[:, 0:64:2], w1[:])
        vector.drain()
        vector.tensor_copy(w12[:, 1:64:2], w2[:])
        vector.drain().then_inc(svp, 1)
        vector.tensor_copy(w1[:], w12[:, 0:64:2])
        vector.drain()
        vector.tensor_copy(w2[:], w12[:, 1:64:2])
        vector.drain()
        # CE: lt = logits[target]
        vector.tensor_copy(tgtf[:], tgt[:])
        vector.drain()
        vector.tensor_scalar(sel[:], tgtf[:], 0.0, None, AO.is_equal)
        vector.drain()
        vector.tensor_tensor(lt[:], lgt[:, 0:96:3], sel[:], AO.mult)
        vector.drain()
        vector.tensor_scalar(sel[:], tgtf[:], 1.0, None, AO.is_equal)
        vector.drain()
        vector.tensor_tensor(pr[:], lgt[:, 1:96:3], sel[:], AO.mult)
        vector.drain()
        vector.tensor_tensor(lt[:], lt[:], pr[:], AO.add)
        vector.drain()
        vector.tensor_scalar(sel[:], tgtf[:], 2.0, None, AO.is_equal)
        vector.drain()
        vector.tensor_tensor(pr[:], lgt[:, 2:96:3], sel[:], AO.mult)
        vector.drain()
        vector.tensor_tensor(lt[:], lt[:], pr[:], AO.add)
        vector.drain()
        vector.wait_ge(sa, 3)  # lnden
        vector.tensor_tensor(ce[:], lnden[:], lt[:], AO.subtract)
        vector.drain()
        vector.tensor_tensor(pr[:], wt[:], ce[:], AO.mult)
        vector.drain()
        # G columns: Sw S1 S2 CE m1 q1 my1 qy1 m2 q2 my2 qy2
        vector.tensor_reduce(G[:, 0:1], wt[:], AX.X, AO.add)
        vector.drain()
        vector.tensor_reduce(G[:, 1:2], w1[:], AX.X, AO.add)
        vector.drain()
        vector.tensor_reduce(G[:, 2:3], w2[:], AX.X, AO.add)
        vector.drain()
        vector.tensor_reduce(G[:, 3:4], pr[:], AX.X, AO.add)
        vector.drain()
        vector.tensor_copy(m1ff[:], m1t[:])
        vector.drain()
        vector.tensor_copy(m2ff[:], m2t[:])
        vector.drain()
        vector.tensor_tensor(pr[:], w1[:], sc_r[:], AO.mult)
        vector.drain()
        vector.tensor_reduce(G[:, 4:5], pr[:], AX.X, AO.add)
        vector.drain()
        vector.tensor_tensor(sq[:], sc_r[:], sc_r[:], AO.mult)
        vector.drain()
        vector.tensor_tensor(pr[:], w1[:], sq[:], AO.mult)
        vector.drain()
        vector.tensor_reduce(G[:, 5:6], pr[:], AX.X, AO.add)
        vector.drain()
        vector.tensor_tensor(pr[:], w1[:], m1ff[:], AO.mult)
        vector.drain()
        vector.tensor_reduce(G[:, 6:7], pr[:], AX.X, AO.add)
        vector.drain()
        vector.tensor_tensor(pr[:], m1ff[:], m1ff[:], AO.mult)
        vector.drain()
        vector.tensor_tensor(pr[:], w1[:], pr[:], AO.mult)
        vector.drain()
        vector.tensor_reduce(G[:, 7:8], pr[:], AX.X, AO.add)
        vector.drain()
        vector.tensor_tensor(pr[:], w2[:], sc_r[:], AO.mult)
        vector.drain()
        vector.tensor_reduce(G[:, 8:9], pr[:], AX.X, AO.add)
        vector.drain()
        vector.tensor_tensor(pr[:], w2[:], sq[:], AO.mult)
        vector.drain()
        vector.tensor_reduce(G[:, 9:10], pr[:], AX.X, AO.add)
        vector.drain()
        vector.tensor_tensor(pr[:], w2[:], m2ff[:], AO.mult)
        vector.drain()
        vector.tensor_reduce(G[:, 10:11], pr[:], AX.X, AO.add)
        vector.drain()
        vector.tensor_tensor(pr[:], m2ff[:], m2ff[:], AO.mult)
        vector.drain()
        vector.tensor_tensor(pr[:], w2[:], pr[:], AO.mult)
        vector.drain()
        vector.tensor_reduce(G[:, 11:12], pr[:], AX.X, AO.add)
        vector.drain().then_inc(svp, 1)
        # w12row for phase 2
        vector.tensor_scalar(mrow_w[:], mrow[:], 0.0, None, AO.is_gt)
        vector.drain()
        vector.tensor_tensor(w12row[:], wrow[:], mrow_w[:], AO.mult)
        vector.drain()
        vector.tensor_copy(w12row_bf[:], w12row[:])
        vector.drain()
        vector.tensor_copy(w12row[:], w12row_bf[:])
        vector.drain()
        # ---- phase 1 loop ----
        vector.wait_ge(dm, 16 * 13)  # xrow in
        for K in range(NB // 2):
            s = K % 3
            if K >= 3:
                vector.wait_ge(sp, K - 1)
            k0, k1 = 2 * K, 2 * K + 1
            vector.tensor_scalar(d_[s][:, 0:M], xrow[:], sc_r[:, k0 : k0 + 1], None, AO.subtract)
            vector.tensor_scalar(d_[s][:, M : 2 * M], xrow[:], sc_r[:, k1 : k1 + 1], None, AO.subtract)
            vector.drain()
            vector.tensor_scalar(
                a_[s][:].bitcast(U16), d_[s][:].bitcast(U16), 0x7FFF, None, AO.bitwise_and
            )
            vector.drain().then_inc(sv, 1)
            vector.wait_ge(sa, 3 + 4 * K + 2)
            vector.tensor_tensor(ab1_[s][:], a_[s][:], b1_[s][:], AO.mult)
            vector.wait_ge(sa, 3 + 4 * K + 4)
            vector.tensor_tensor(ab2_[s][:], a_[s][:], b2_[s][:], AO.mult)
            vector.drain().then_inc(sv, 1)
        # ---- phase 2 ----
        vector.wait_ge(sp, NB // 2 + 1)
        vector.tensor_copy(arsb[:], ps_a[:])
        vector.drain()
        vector.tensor_tensor(t_[:], arsb[:], w12row[:], AO.mult)
        vector.drain()
        vector.tensor_reduce(sc2[:, 0:1], t_[:], AX.X, AO.add)  # g_a
        vector.drain()
        vector.tensor_tensor(t2_[:], t_[:], arsb[:], AO.mult)
        vector.drain()
        vector.tensor_reduce(sc2[:, 1:2], t2_[:], AX.X, AO.add)  # T1aa
        vector.drain()
        vector.wait_ge(sa2, 4)
        vector.wait_ge(dm, 16 * 15)
        vector.tensor_tensor(t2_[:], V[:, 0:M], w12row[:], AO.mult)
        vector.drain()
        vector.tensor_reduce(sc2[:, 2:3], t2_[:], AX.X, AO.add)  # g_b
        vector.drain()
        vector.tensor_tensor(t2_[:], t2_[:], V[:, 0:M], AO.mult)
        vector.drain()
        vector.tensor_reduce(sc2[:, 3:4], t2_[:], AX.X, AO.add)  # T1bb
        vector.drain()
        vector.tensor_tensor(t2_[:], t_[:], V[:, 0:M], AO.mult)
        vector.drain()
        vector.tensor_reduce(sc2[:, 4:5], t2_[:], AX.X, AO.add)  # T1ab
        vector.drain()
        vector.tensor_tensor(t2_[:], V[:, M : 2 * M], w12row[:], AO.mult)
        vector.drain()
        vector.tensor_reduce(sc2[:, 5:6], t2_[:], AX.X, AO.add)  # S_ab
        vector.drain()
        vector.tensor_copy(Gsb[:], ps_g[:])
        vector.drain().then_inc(sv, 200)

    @block.tensor
    def _(tensor):
        tensor.wait_ge(svp, 5)  # G ready
        tensor.matmul(ps_g[:], ones_t[:], G[:], start=True, stop=True).then_inc(sp, 1)
        for K in range(NB // 2):
            s = K % 3
            st = K == 0
            last = K == NB // 2 - 1
            k0, k1 = 2 * K, 2 * K + 1
            lw0 = w12[:, 2 * k0 : 2 * k0 + 2]
            lw1 = w12[:, 2 * k1 : 2 * k1 + 2]
            tensor.wait_ge(sa, 3 + 4 * K + 4)
            tensor.matmul(ps_b1[:], lw0, b1_[s][:, 0:M], start=st, stop=False)
            tensor.matmul(ps_b1[:], lw1, b1_[s][:, M : 2 * M], start=False, stop=last)
            tensor.matmul(ps_b2[:], lw0, b2_[s][:, 0:M], start=st, stop=False)
            tensor.matmul(ps_b2[:], lw1, b2_[s][:, M : 2 * M], start=False, stop=last)
            tensor.wait_ge(sv, 2 * K + 2)
            tensor.matmul(ps_a[:], lw0, a_[s][:, 0:M], start=st, stop=False)
            tensor.matmul(ps_a[:], lw1, a_[s][:, M : 2 * M], start=False, stop=last)
            tensor.matmul(ps_p1[:], lw0, ab1_[s][:, 0:M], start=st, stop=False)
            tensor.matmul(ps_p1[:], lw1, ab1_[s][:, M : 2 * M], start=False, stop=last)
            tensor.matmul(ps_p2[:], lw0, ab2_[s][:, 0:M], start=st, stop=False)
            tensor.matmul(ps_p2[:], lw1, ab2_[s][:, M : 2 * M], start=False, stop=last).then_inc(sp, 1)

    return nc, es


_NC_CACHE = {}


def kernel(logits, target, weight, mass1, mass2):
    logits = np.asarray(logits, dtype=np.float32)
    target_i = np.asarray(target).astype(np.int32)
    weight = np.asarray(weight, dtype=np.float32)
    mass1 = np.asarray(mass1, dtype=np.float32)
    mass2 = np.asarray(mass2, dtype=np.float32)
    m1b = mass1.astype(ml_dtypes.bfloat16)
    m2b = mass2.astype(ml_dtypes.bfloat16)

    if "nc" not in _NC_CACHE:
        _NC_CACHE["nc"] = _build_program()
    nc, _ = _NC_CACHE["nc"]

    in_maps = []
    for c in range(NCORES):
        sl = slice(c * M, (c + 1) * M)
        in_maps.append(
            {
                "lg": logits,
                "lgm": np.ascontiguousarray(logits[sl]),
                "tg": target_i,
                "wf": weight,
                "wm": weight[sl].reshape(1, M),
                "m1f": m1b,
                "m2f": m2b,
                "m1m": m1b[sl].reshape(1, M),
                "m2m": m2b[sl].reshape(1, M),
            }
        )
    res = run_bass_kernel_spmd(nc, in_maps, list(range(NCORES)))
    outs = [r["out"] for r in res.results]
    return _combine(outs)


def _combine(outs):
    G = np.asarray(outs[0][0:12], dtype=np.float64)
    Sw, S1, S2, CEs = G[0], G[1], G[2], G[3]
    m1, q1, my1, qy1 = G[4], G[5], G[6], G[7]
    m2, q2, my2, qy2 = G[8], G[9], G[10], G[11]
    # per-core partials: sc2 [2,8] flattened at out[12:28]
    P = np.zeros((2, 8), dtype=np.float64)
    for o in outs:
        P += o[12:28].reshape(2, 8).astype(np.float64)
    ce_mean = CEs / max(Sw, EPS_W)

    def disco(row, Sr, m, q, my, qy):
        g_a, T1aa, g_b, T1bb, T1ab, S_ab = P[row, 0:6]
        s = 1.0 / max(Sr, EPS_W)
        dcov = s * s * S_ab - 2.0 * s**3 * T1ab + s**4 * g_a * g_b
        dvx = 2.0 * (s * q - (s * m) ** 2) - 2.0 * s**3 * T1aa + (s * s * g_a) ** 2
        dvy = 2.0 * (s * qy - (s * my) ** 2) - 2.0 * s**3 * T1bb + (s * s * g_b) ** 2
        ok = (dvx > EPS_VAR) and (dvy > EPS_VAR)
        if not ok:
            return 0.0
        return np.sqrt(np.abs(dcov) / np.sqrt(dvx * dvy))

    d1 = disco(0, S1, m1, q1, my1, qy1)
    d2 = disco(1, S2, m2, q2, my2, qy2)
    return np.float32(ce_mean + DISCO_LAMBDA * (d1 + d2))



# revision 48
# speedup vs baseline: 2151.9923x; 2151.9923x over previous
"""DiSco weighted loss kernel for 8 trn2 NeuronCores.

Math: for symmetric a_ij=|x_i-x_j|, the weighted distance-correlation terms
collapse to  dcov = S_ab - 2*T1ab + g_a*g_b  with
  ar_i = sum_j w_j a_ij,  g_a = sum_i w_i ar_i,  T1ab = sum_i w_i ar_i br_i,
  S_ab = sum_ij w_i w_j a_ij b_ij,
and dvar_x = 2(q - m^2) - 2*T1aa + g_a^2 exactly (|.|^2 loses the abs).
Each core owns 512 i-rows (free axis) and scans all j (partition axis,
32 column-sets of its [128,32] f-major tiles); TensorE reduces over j via
bf16 matmuls accumulated in PSUM; the per-core scalar partials are summed
on the host (8x22 floats) to avoid a ~20us AllReduce latency floor.

Engine split: ACT builds the b=|dy| tiles (Abs activation, per-partition
bias) starting as soon as the masses land; DVE builds a=|dx| (4x
tensor_scalar + bitwise-and) and the a*b products (2x tensor_tensor); PE
accumulates five weighted row-sum streams in PSUM; GPSIMD (otherwise
idle) does all the prep (ny/masks/w12/w12row) and the CE + moment
statistics concurrently with the pairwise loop, so DVE's tail is just the
six fused phase-2 dot products.  The scr->xrow gather/broadcast DMAs are
issued from DVE's own queue to skip the SP queue's issue backlog.

Dispatch: the compiled executable (jax.jit of the bass_exec custom call,
sharded over the 8 cores) is cached in-process, so warm kernel() calls are
a single PJRT dispatch instead of a re-trace + re-lower every call.

`_build_program(reps)` emits the program `reps` times with per-iteration
semaphore-threshold offsets; reps>1 exists for in-NEFF repeat timing
(test.py measures the slope over reps to isolate device execution time
from the host/network dispatch floor).
"""

from contextlib import ExitStack

import numpy as np
import ml_dtypes

import concourse.bass as bass
from concourse import mybir

F32 = mybir.dt.float32
BF16 = mybir.dt.bfloat16
I32 = mybir.dt.int32
U16 = mybir.dt.uint16
AO = mybir.AluOpType
AF = mybir.ActivationFunctionType
AX = mybir.AxisListType

N, C, NCORES = 4096, 3, 8
M = N // NCORES  # 512 rows per core
NB = 32  # j-sets (columns of the [128,32] tiles)
LND = 6  # ACT loop group after which lnden is emitted

DISCO_LAMBDA = 0.1
EPS_W = 1e-8
EPS_VAR = 1e-10


def _build_program(reps=1, nb=NB):
    nc = bass.Bass()
    lg = nc.declare_dram_parameter("lg", [N, C], F32, isOutput=False)
    lgm = nc.declare_dram_parameter("lgm", [M, C], F32, isOutput=False)
    tg = nc.declare_dram_parameter("tg", [N], I32, isOutput=False)
    wf = nc.declare_dram_parameter("wf", [N], F32, isOutput=False)
    wm = nc.declare_dram_parameter("wm", [1, M], F32, isOutput=False)
    m1f = nc.declare_dram_parameter("m1f", [N], BF16, isOutput=False)
    m2f = nc.declare_dram_parameter("m2f", [N], BF16, isOutput=False)
    m1m = nc.declare_dram_parameter("m1m", [1, M], BF16, isOutput=False)
    m2m = nc.declare_dram_parameter("m2m", [1, M], BF16, isOutput=False)
    idn_d = nc.declare_dram_parameter("idn", [128, 128], BF16, isOutput=False)
    w4_d = nc.declare_dram_parameter("w4", [4, 512], BF16, isOutput=False)
    out = nc.declare_dram_parameter("out", [32], F32, isOutput=True)

    es = ExitStack()
    def sb(name, shp, dt):
        return es.enter_context(nc.sbuf_tensor(name, shp, dt))

    def ps(name, shp):
        return es.enter_context(nc.psum_tensor(name, shp, F32))

    lgt = sb("lgt", [128, 96], F32)
    lgtm = sb("lgtm", [128, 12], F32)
    tgt = sb("tgt", [128, 32], I32)
    wt = sb("wt", [128, 32], F32)
    m1t = sb("m1t", [128, 32], BF16)
    m2t = sb("m2t", [128, 32], BF16)
    wrow = sb("wrow", [2, M], F32)
    mrow = sb("mrow", [2, M], BF16)
    y1row = sb("y1row", [128, M], BF16)
    y2row = sb("y2row", [128, M], BF16)
    xrow = sb("xrow", [128, M], BF16)

    e = sb("e", [128, 96], F32)
    den = sb("den", [128, 32], F32)
    rec = sb("rec", [128, 32], F32)
    sc = sb("sc", [128, 32], F32)
    scbf = sb("scbf", [128, 32], BF16)
    sc_r = sb("sc_r", [128, 32], F32)  # bf16-rounded scores back in f32
    em = sb("em", [128, 12], F32)
    denm = sb("denm", [128, 4], F32)
    recm = sb("recm", [128, 4], F32)
    scm = sb("scm", [128, 4], F32)
    scmbf = sb("scmbf", [128, 4], BF16)
    ny1 = sb("ny1", [128, 32], F32)
    ny2 = sb("ny2", [128, 32], F32)
    msk1 = sb("msk1", [128, 32], F32)
    msk2 = sb("msk2", [128, 32], F32)
    w1 = sb("w1", [128, 32], F32)
    w2 = sb("w2", [128, 32], F32)
    w12 = sb("w12", [128, 64], BF16)
    m1ff = sb("m1ff", [128, 32], F32)
    m2ff = sb("m2ff", [128, 32], F32)
    sq = sb("sq", [128, 32], F32)
    pr = sb("pr", [128, 32], F32)
    pra = sb("pra", [128, 32], F32)
    tgtf = sb("tgtf", [128, 32], F32)
    lt = sb("lt", [128, 32], F32)
    lnden = sb("lnden", [128, 32], F32)
    ce = sb("ce", [128, 32], F32)
    G = sb("G", [128, 12], F32)
    Gsb = sb("Gsb", [1, 12], F32)
    ones_t = sb("ones_t", [128, 1], F32)
    idn = sb("idn_s", [128, 128], BF16)
    W4 = sb("w4_s", [4, 512], BF16)
    T4sb = sb("T4sb", [4, 128], BF16)

    DEPTH = 6  # loop-tile ring depth (PE may lag this many groups)
    d_ = [sb(f"d{i}", [128, 2 * M], BF16) for i in range(DEPTH)]
    a_ = [sb(f"a{i}", [128, 2 * M], BF16) for i in range(DEPTH)]
    b1_ = [sb(f"b1{i}", [128, 2 * M], BF16) for i in range(DEPTH)]
    b2_ = [sb(f"b2{i}", [128, 2 * M], BF16) for i in range(DEPTH)]
    ab1_ = [sb(f"ab1{i}", [128, 2 * M], BF16) for i in range(DEPTH)]
    ab2_ = [sb(f"ab2{i}", [128, 2 * M], BF16) for i in range(DEPTH)]

    mrow_w = sb("mrow_w", [2, M], F32)
    w12row = sb("w12row", [2, M], F32)
    arsb = sb("arsb", [2, M], F32)
    V = sb("V", [2, 2 * M], F32)
    t_ = sb("t_", [2, M], F32)
    t2_ = sb("t2_", [2, M], F32)
    t3_ = sb("t3_", [2, M], F32)
    sc2 = sb("sc2", [2, 8], F32)
    w12row_bf = sb("w12row_bf", [2, M], BF16)
    st_b2 = sb("st_b2", [2, M], F32)
    st_p2 = sb("st_p2", [2, M], F32)

    ps_a = ps("ps_a", [2, M])
    ps_b1 = ps("ps_b1", [2, M])
    ps_b2 = ps("ps_b2", [2, M])
    ps_p1 = ps("ps_p1", [2, M])
    ps_p2 = ps("ps_p2", [2, M])
    ps_g = ps("ps_g", [1, 12])
    ps_t = es.enter_context(nc.psum_tensor("ps_t", [4, 128], BF16))  # transposed own-scores
    ps_x = ps("ps_x", [128, M])   # broadcast own-scores

    dmA = es.enter_context(nc.semaphore("dmA"))    # lgt, lgtm
    dmB = es.enter_context(nc.semaphore("dmB"))    # m1t, m2t, wt
    dmC = es.enter_context(nc.semaphore("dmC"))    # y1row, y2row (Pool queue)
    dmD = es.enter_context(nc.semaphore("dmD"))    # tgt, wrow, mrow x2
    sa = es.enter_context(nc.semaphore("sa"))      # ACT progress
    svp = es.enter_context(nc.semaphore("svp"))    # DVE score-chain progress
    sv = es.enter_context(nc.semaphore("sv"))      # DVE loop progress
    sp = es.enter_context(nc.semaphore("sp"))      # PE progress
    sa2 = es.enter_context(nc.semaphore("sa2"))    # ACT V copies
    spool = es.enter_context(nc.semaphore("spool"))  # Pool progress
    spb = es.enter_context(nc.semaphore("spb"))      # PE b-part progress
    spx = es.enter_context(nc.semaphore("spx"))      # PE score-broadcast progress
    dmV = es.enter_context(nc.semaphore("dmV"))      # V row-1 DMA moves
    dmE = es.enter_context(nc.semaphore("dmE"))      # idn/w4 constant loads
    block = es.enter_context(nc.Block())

    # per-iteration semaphore increments
    DMA_IT, DMB_IT, DMC_IT, DMD_IT = 32, 48, 32, 64
    SPX_IT = 2
    SA_IT = 3 + 4 * (nb // 2)
    SVP_IT = 6
    SV_IT = 2 * (nb // 2) + 200
    SP_IT = 1 + nb // 2
    SA2_IT = 4
    SPOOL_IT = 3

    def sa_val(K, v):
        # sa value after the v-th activation of loop group K (v=1..4)
        return 2 + 4 * K + v + (1 if K > LND else 0)

    for r in range(reps):
        DMAB, DMBB, DMCB, DMDB = DMA_IT * r, DMB_IT * r, DMC_IT * r, DMD_IT * r
        SPXB = SPX_IT * r
        DMVB = 64 * r
        SAB = SA_IT * r
        SVPB = SVP_IT * r
        SVB = SV_IT * r
        SPB = SP_IT * r
        SA2B = SA2_IT * r
        SPOOLB = SPOOL_IT * r
        SPBB = (nb // 2) * r

        @block.sync
        def _(sync):
            sync.dma_start(out=lgtm[:].rearrange("p (q c) -> p q c", c=3), in_=lgm[:].rearrange("(q p) c -> p q c", q=4)).then_inc(dmA, 16)
            sync.dma_start(out=lgt[:], in_=lg[:].rearrange("(p f) c -> p (f c)", f=32)).then_inc(dmA, 16)
            if r == 0:
                sync.dma_start(out=idn[:], in_=idn_d[:]).then_inc(dmE, 16)
                sync.dma_start(out=W4[:], in_=w4_d[:]).then_inc(dmE, 16)
            sync.dma_start(out=m1t[:], in_=m1f[:].rearrange("(p f) -> p f", f=32)).then_inc(dmB, 16)
            sync.dma_start(out=m2t[:], in_=m2f[:].rearrange("(p f) -> p f", f=32)).then_inc(dmB, 16)
            sync.dma_start(out=wt[:], in_=wf[:].rearrange("(p f) -> p f", f=32)).then_inc(dmB, 16)
            sync.dma_start(out=y1row[:], in_=m1m[:].broadcast_to([128, M])).then_inc(dmC, 16)
            sync.dma_start(out=y2row[:], in_=m2m[:].broadcast_to([128, M])).then_inc(dmC, 16)
            sync.dma_start(out=tgt[:], in_=tg[:].rearrange("(p f) -> p f", f=32)).then_inc(dmD, 16)
            sync.dma_start(out=wrow[:], in_=wm[:].broadcast_to([2, M])).then_inc(dmD, 16)
            sync.dma_start(out=mrow[0:1, :], in_=m1m[:]).then_inc(dmD, 16)
            sync.dma_start(out=mrow[1:2, :], in_=m2m[:]).then_inc(dmD, 16)
            # V row-1 moves
            sync.wait_ge(sa2, SA2B + 2)
            sync.dma_start(out=V[1:2, 0:M], in_=st_b2[1:2, :]).then_inc(dmV, 16)
            sync.wait_ge(sa2, SA2B + 4)
            sync.dma_start(out=V[1:2, M : 2 * M], in_=st_p2[1:2, :]).then_inc(dmV, 16)
            # outputs
            sync.wait_ge(sv, SVB + SV_IT)
            sync.dma_start(out=out[0:12], in_=Gsb[:]).then_inc(dmV, 16)
            sync.dma_start(out=out[12:28], in_=sc2[:]).then_inc(dmV, 16)

        @block.gpsimd
        def _(gp):
            gp.memset(ones_t[:], 1.0)

        @block.scalar
        def _(scalar):
            scalar.wait_ge(dmA, DMAB + 16)
            scalar.activation(em[:], lgtm[:], AF.Exp).then_inc(sa, 1)
            scalar.wait_ge(dmA, DMAB + 32)
            scalar.activation(e[:], lgt[:], AF.Exp).then_inc(sa, 1)
            # b tiles: start as soon as ny1/ny2 + y rows are in
            scalar.wait_ge(svp, SVPB + 1)
            scalar.wait_ge(dmC, DMCB + 32)
            for K in range(nb // 2):
                s = K % DEPTH
                spw = SPBB + K - DEPTH + 1
                if spw > 0:
                    scalar.wait_ge(spb, spw)
                k0, k1 = 2 * K, 2 * K + 1
                scalar.activation(b1_[s][:, 0:M], y1row[:], AF.Abs, bias=ny1[:, k0 : k0 + 1]).then_inc(sa, 1)
                scalar.activation(b1_[s][:, M : 2 * M], y1row[:], AF.Abs, bias=ny1[:, k1 : k1 + 1]).then_inc(sa, 1)
                scalar.activation(b2_[s][:, 0:M], y2row[:], AF.Abs, bias=ny2[:, k0 : k0 + 1]).then_inc(sa, 1)
                scalar.activation(b2_[s][:, M : 2 * M], y2row[:], AF.Abs, bias=ny2[:, k1 : k1 + 1]).then_inc(sa, 1)
                if K == LND:
                    scalar.wait_ge(svp, SVPB + 4)  # den
                    scalar.activation(lnden[:], den[:], AF.Ln).then_inc(sa, 1)
            if nb // 2 <= LND:
                scalar.wait_ge(svp, SVPB + 4)
                scalar.activation(lnden[:], den[:], AF.Ln).then_inc(sa, 1)
            # V assembly: partition-0 copies; row-1 halves moved by SP DMAs
            scalar.wait_ge(spb, SPBB + nb // 2)
            scalar.activation(V[0:1, 0:M], ps_b1[0:1, :], AF.Copy).then_inc(sa2, 1)
            scalar.activation(st_b2[:], ps_b2[:], AF.Copy).then_inc(sa2, 1)
            scalar.wait_ge(sp, SPB + nb // 2)
            scalar.activation(V[0:1, M : 2 * M], ps_p1[0:1, :], AF.Copy).then_inc(sa2, 1)
            scalar.activation(st_p2[:], ps_p2[:], AF.Copy).then_inc(sa2, 1)

        @block.vector
        def _(vector):
            # prep: negated masses, masked weights, bf16 weight pairs
            vector.wait_ge(dmB, DMBB + 48)
            vector.tensor_scalar(ny1[:], m1t[:], -1.0, None, AO.mult)
            vector.tensor_scalar(ny2[:], m2t[:], -1.0, None, AO.mult)
            vector.drain().then_inc(svp, 1)
            vector.tensor_scalar(msk1[:], m1t[:], 0.0, None, AO.is_gt)
            vector.tensor_scalar(msk2[:], m2t[:], 0.0, None, AO.is_gt)
            vector.tensor_tensor(w1[:], wt[:], msk1[:], AO.mult)
            vector.tensor_tensor(w2[:], wt[:], msk2[:], AO.mult)
            vector.drain()
            vector.tensor_copy(w12[:, 0:64:2], w1[:])
            vector.tensor_copy(w12[:, 1:64:2], w2[:])
            vector.drain().then_inc(svp, 1)
            # own-row softmax scores (gate the transpose/broadcast chain)
            vector.wait_ge(sa, SAB + 1)
            vector.tensor_reduce(denm[:], em[:].rearrange("p (f c) -> p f c", c=3), AX.X, AO.add)
            vector.drain()
            vector.reciprocal(recm[:], denm[:])
            vector.drain()
            vector.tensor_tensor(scm[:], em[:, 0:12:3], recm[:], AO.mult)
            vector.drain()
            vector.tensor_copy(scmbf[:], scm[:]).then_inc(svp, 1)
            # full-N scores
            vector.wait_ge(sa, SAB + 2)
            vector.tensor_reduce(den[:], e[:].rearrange("p (f c) -> p f c", c=3), AX.X, AO.add)
            vector.drain()
            vector.reciprocal(rec[:], den[:])
            vector.drain()
            vector.tensor_tensor(sc[:], e[:, 0:96:3], rec[:], AO.mult)
            vector.drain()
            vector.tensor_copy(scbf[:], sc[:])
            vector.drain()
            vector.tensor_copy(sc_r[:], scbf[:]).then_inc(svp, 1)
            # own-score broadcast: PSUM transpose -> SBUF -> PSUM bcast -> xrow
            vector.wait_ge(spx, SPXB + 1)
            vector.tensor_copy(T4sb[:], ps_t[:]).then_inc(svp, 1)
            vector.wait_ge(spx, SPXB + 2)
            vector.tensor_copy(xrow[:], ps_x[:])
            # ---- pairwise loop, with CE/G statistics interleaved into
            # the per-iteration slack (DVE is ~0.5us/group lighter than ACT) ----
            def stats(K):
                if K == 1:
                    vector.wait_ge(dmD, DMDB + 64)
                    vector.tensor_copy(tgtf[:], tgt[:])
                    vector.tensor_copy(m1ff[:], m1t[:])
                elif K == 2:
                    vector.tensor_copy(m2ff[:], m2t[:])
                    vector.tensor_tensor(sq[:], sc_r[:], sc_r[:], AO.mult)
                elif K == 3:
                    vector.scalar_tensor_tensor(lt[:], tgtf[:], 0.0, lgt[:, 0:96:3], AO.is_equal, AO.mult)
                    vector.scalar_tensor_tensor(pr[:], tgtf[:], 1.0, lgt[:, 1:96:3], AO.is_equal, AO.mult)
                elif K == 4:
                    vector.tensor_tensor(lt[:], lt[:], pr[:], AO.add)
                    vector.scalar_tensor_tensor(pr[:], tgtf[:], 2.0, lgt[:, 2:96:3], AO.is_equal, AO.mult)
                elif K == 5:
                    vector.tensor_tensor(lt[:], lt[:], pr[:], AO.add)
                    vector.tensor_scalar(mrow_w[:], mrow[:], 0.0, None, AO.is_gt)
                elif K == 6:
                    vector.tensor_tensor(w12row[:], wrow[:], mrow_w[:], AO.mult)
                elif K == 7:
                    vector.tensor_copy(w12row_bf[:], w12row[:])
                elif K == 8:
                    vector.tensor_copy(w12row[:], w12row_bf[:])
                    vector.wait_ge(sa, SAB + (sa_val(LND, 4) + 1 if nb // 2 > LND else SA_IT))  # lnden
                    vector.tensor_tensor(ce[:], lnden[:], lt[:], AO.subtract)
                elif K == 9:
                    vector.scalar_tensor_tensor(pr[:], wt[:], 0.0, wt[:], AO.is_ge, AO.mult, accum_out=G[:, 0:1])
                    vector.scalar_tensor_tensor(pr[:], wt[:], 1.0, msk1[:], AO.mult, AO.mult, accum_out=G[:, 1:2])
                elif K == 10:
                    vector.scalar_tensor_tensor(pr[:], wt[:], 1.0, msk2[:], AO.mult, AO.mult, accum_out=G[:, 2:3])
                    vector.scalar_tensor_tensor(pr[:], wt[:], 1.0, ce[:], AO.mult, AO.mult, accum_out=G[:, 3:4])
                elif K == 11:
                    vector.scalar_tensor_tensor(pr[:], w1[:], 1.0, sc_r[:], AO.mult, AO.mult, accum_out=G[:, 4:5])
                    vector.scalar_tensor_tensor(pr[:], w1[:], 1.0, sq[:], AO.mult, AO.mult, accum_out=G[:, 5:6])
                elif K == 12:
                    vector.scalar_tensor_tensor(pra[:], w1[:], 1.0, m1ff[:], AO.mult, AO.mult, accum_out=G[:, 6:7])
                    vector.scalar_tensor_tensor(pr[:], w2[:], 1.0, sc_r[:], AO.mult, AO.mult, accum_out=G[:, 8:9])
                elif K == 13:
                    vector.scalar_tensor_tensor(pr[:], pra[:], 1.0, m1ff[:], AO.mult, AO.mult, accum_out=G[:, 7:8])
                    vector.scalar_tensor_tensor(pr[:], w2[:], 1.0, sq[:], AO.mult, AO.mult, accum_out=G[:, 9:10])
                elif K == 14:
                    vector.scalar_tensor_tensor(pra[:], w2[:], 1.0, m2ff[:], AO.mult, AO.mult, accum_out=G[:, 10:11])
                elif K == 15:
                    vector.scalar_tensor_tensor(pr[:], pra[:], 1.0, m2ff[:], AO.mult, AO.mult, accum_out=G[:, 11:12])
                    vector.drain().then_inc(svp, 1)

            for K in range(nb // 2):
                s = K % DEPTH
                spw = SPB + K - DEPTH + 1 if K >= DEPTH else SPB + K - DEPTH
                if spw > 0:
                    vector.wait_ge(sp, spw)
                if nb == NB:
                    stats(K)
                k0, k1 = 2 * K, 2 * K + 1
                vector.tensor_scalar(d_[s][:, 0:M], xrow[:], sc_r[:, k0 : k0 + 1], None, AO.subtract)
                vector.tensor_scalar(d_[s][:, M : 2 * M], xrow[:], sc_r[:, k1 : k1 + 1], None, AO.subtract)
                vector.tensor_scalar(
                    a_[s][:].bitcast(U16), d_[s][:].bitcast(U16), 0x7FFF, None, AO.bitwise_and
                ).then_inc(sv, 1)
                vector.wait_ge(sa, SAB + sa_val(K, 2))
                vector.tensor_tensor(ab1_[s][:], a_[s][:], b1_[s][:], AO.mult)
                vector.wait_ge(sa, SAB + sa_val(K, 4))
                vector.tensor_tensor(ab2_[s][:], a_[s][:], b2_[s][:], AO.mult).then_inc(sv, 1)
            if nb != NB:
                # timing variants: emit all stats after the loop
                for K in range(16):
                    stats(K)
            # ---- phase 2: fused dot products via stt-accum ----
            vector.wait_ge(sp, SPB + nb // 2)
            vector.tensor_copy(arsb[:], ps_a[:])
            vector.drain()
            vector.scalar_tensor_tensor(t_[:], arsb[:], 1.0, w12row[:], AO.mult, AO.mult, accum_out=sc2[:, 0:1])   # g_a
            vector.scalar_tensor_tensor(t3_[:], t_[:], 1.0, arsb[:], AO.mult, AO.mult, accum_out=sc2[:, 1:2])      # T1aa
            vector.wait_ge(sa2, SA2B + 1)
            vector.wait_ge(dmV, DMVB + 16)
            vector.scalar_tensor_tensor(t3_[:], t_[:], 1.0, V[:, 0:M], AO.mult, AO.mult, accum_out=sc2[:, 4:5])    # T1ab
            vector.scalar_tensor_tensor(t2_[:], V[:, 0:M], 1.0, w12row[:], AO.mult, AO.mult, accum_out=sc2[:, 2:3])  # g_b
            vector.scalar_tensor_tensor(t3_[:], t2_[:], 1.0, V[:, 0:M], AO.mult, AO.mult, accum_out=sc2[:, 3:4])   # T1bb
            vector.wait_ge(sa2, SA2B + 3)
            vector.wait_ge(dmV, DMVB + 32)
            vector.scalar_tensor_tensor(t3_[:], V[:, M : 2 * M], 1.0, w12row[:], AO.mult, AO.mult, accum_out=sc2[:, 5:6])  # S_ab
            vector.wait_ge(sp, SPB + nb // 2 + 1)
            vector.tensor_copy(Gsb[:], ps_g[:]).then_inc(sv, 200)

        @block.tensor
        def _(tensor):
            def bpart(J):
                s = J % DEPTH
                st = J == 0
                last = J == nb // 2 - 1
                k0, k1 = 2 * J, 2 * J + 1
                lw0 = w12[:, 2 * k0 : 2 * k0 + 2]
                lw1 = w12[:, 2 * k1 : 2 * k1 + 2]
                tensor.wait_ge(sa, SAB + sa_val(J, 4))
                tensor.matmul(ps_b1[:], lw0, b1_[s][:, 0:M], start=st, stop=False)
                tensor.matmul(ps_b2[:], lw0, b2_[s][:, 0:M], start=st, stop=False)
                tensor.matmul(ps_b1[:], lw1, b1_[s][:, M : 2 * M], start=False, stop=last)
                tensor.matmul(ps_b2[:], lw1, b2_[s][:, M : 2 * M], start=False, stop=last).then_inc(spb, 1)

            def abpart(K):
                s = K % DEPTH
                st = K == 0
                last = K == nb // 2 - 1
                k0, k1 = 2 * K, 2 * K + 1
                lw0 = w12[:, 2 * k0 : 2 * k0 + 2]
                lw1 = w12[:, 2 * k1 : 2 * k1 + 2]
                tensor.wait_ge(sv, SVB + 2 * K + 2)
                tensor.matmul(ps_a[:], lw0, a_[s][:, 0:M], start=st, stop=False)
                tensor.matmul(ps_p1[:], lw0, ab1_[s][:, 0:M], start=st, stop=False)
                tensor.matmul(ps_p2[:], lw0, ab2_[s][:, 0:M], start=st, stop=False)
                tensor.matmul(ps_a[:], lw1, a_[s][:, M : 2 * M], start=False, stop=last)
                tensor.matmul(ps_p1[:], lw1, ab1_[s][:, M : 2 * M], start=False, stop=last)
                tensor.matmul(ps_p2[:], lw1, ab2_[s][:, M : 2 * M], start=False, stop=last).then_inc(sp, 1)

            # own-score transpose + broadcast (idn/W4 are constant inputs)
            tensor.wait_ge(dmE, 32)
            tensor.wait_ge(svp, SVPB + 3)
            tensor.transpose(ps_t[:], scmbf[:], idn[:]).then_inc(spx, 1)
            tensor.wait_ge(svp, SVPB + 5)
            for c in range(4):
                mm = tensor.matmul(ps_x[:, 128 * c : 128 * (c + 1)], W4[:, 128 * c : 128 * (c + 1)], T4sb[:], start=True, stop=True)
            mm.then_inc(spx, 1)
            tensor.wait_ge(svp, SVPB + 2)  # w12 ready
            bpart(0)
            for K in range(nb // 2):
                if K + 1 < nb // 2:
                    bpart(K + 1)
                abpart(K)
            tensor.wait_ge(svp, SVPB + 6)  # G ready
            tensor.matmul(ps_g[:], ones_t[:], G[:], start=True, stop=True).then_inc(sp, 1)

    return nc, es


_NC_CACHE = {}


def _get_runner(reps=1, nb=NB):
    """Build (once) and cache the program + AOT-jitted single-dispatch callable."""
    key = ("runner", reps, nb)
    if key in _NC_CACHE:
        return _NC_CACHE[key]

    import jax
    from jax.sharding import Mesh, PartitionSpec
    from jax.experimental.shard_map import shard_map
    import concourse.bass2jax as b2j

    nc, _es = _build_program(reps, nb)
    b2j.install_neuronx_cc_hook()

    partition_name = nc.partition_id_tensor.name if nc.partition_id_tensor else None
    in_names, out_names, out_avals, out_shapes = [], [], [], []
    for alloc in nc.m.functions[0].allocations:
        if not isinstance(alloc, mybir.MemoryLocationSet):
            continue
        name = alloc.memorylocations[0].name
        if alloc.kind == "ExternalInput":
            if name != partition_name:
                in_names.append(name)
        elif alloc.kind == "ExternalOutput":
            out_names.append(name)
            shape = tuple(alloc.tensor_shape)
            dtype = mybir.dt.np(alloc.dtype)
            out_avals.append(jax.core.ShapedArray(shape, dtype))
            out_shapes.append((shape, dtype))
    n_params = len(in_names)
    n_outs = len(out_names)
    in_names_all = list(in_names) + list(out_names)
    if partition_name is not None:
        in_names_all.append(partition_name)

    def _body(*args):
        operands = list(args)
        if partition_name is not None:
            operands.append(b2j.partition_id_tensor())
        outs = b2j._bass_exec_p.bind(
            *operands,
            out_avals=tuple(out_avals),
            in_names=tuple(in_names_all),
            out_names=tuple(out_names),
            lowering_input_output_aliases=(),
            sim_require_finite=True,
            sim_require_nnan=True,
            nc=nc,
        )
        return tuple(outs)

    devices = jax.devices()[:NCORES]
    mesh = Mesh(np.asarray(devices), ("core",))
    in_specs = (PartitionSpec("core"),) * (n_params + n_outs)
    out_specs = (PartitionSpec("core"),) * n_outs
    jitted = jax.jit(
        shard_map(_body, mesh=mesh, in_specs=in_specs, out_specs=out_specs, check_rep=False),
        keep_unused=True,
    )

    def run(in_maps):
        concat_in = [
            np.concatenate([np.asarray(in_maps[c][name]) for c in range(NCORES)], axis=0)
            for name in in_names
        ]
        zeros = [np.zeros((NCORES * s[0], *s[1:]), d) for s, d in out_shapes]
        outs = jitted(*concat_in, *zeros)
        jax.block_until_ready(outs)
        return [
            {
                name: np.asarray(outs[i]).reshape(NCORES, *out_shapes[i][0])[c]
                for i, name in enumerate(out_names)
            }
            for c in range(NCORES)
        ]

    _NC_CACHE[key] = run
    return run


_IDN = np.eye(128, dtype=ml_dtypes.bfloat16)
_W4 = np.zeros((4, 512), dtype=ml_dtypes.bfloat16)
for _c in range(4):
    _W4[_c, 128 * _c : 128 * (_c + 1)] = 1


def _make_in_maps(logits, target_i, weight, m1b, m2b):
    in_maps = []
    for c in range(NCORES):
        sl = slice(c * M, (c + 1) * M)
        in_maps.append(
            {
                "idn": _IDN,
                "w4": _W4,
                "lg": logits,
                "lgm": np.ascontiguousarray(logits[sl]),
                "tg": target_i,
                "wf": weight,
                "wm": weight[sl].reshape(1, M),
                "m1f": m1b,
                "m2f": m2b,
                "m1m": m1b[sl].reshape(1, M),
                "m2m": m2b[sl].reshape(1, M),
            }
        )
    return in_maps


def kernel(logits, target, weight, mass1, mass2):
    logits = np.asarray(logits, dtype=np.float32)
    target_i = np.asarray(target).astype(np.int32)
    weight = np.asarray(weight, dtype=np.float32)
    mass1 = np.asarray(mass1, dtype=np.float32)
    mass2 = np.asarray(mass2, dtype=np.float32)
    m1b = mass1.astype(ml_dtypes.bfloat16)
    m2b = mass2.astype(ml_dtypes.bfloat16)

    run = _get_runner(reps=1)
    res = run(_make_in_maps(logits, target_i, weight, m1b, m2b))
    outs = [r["out"] for r in res]
    return _combine(outs)


def _combine(outs):
    G = np.asarray(outs[0][0:12], dtype=np.float64)
    Sw, S1, S2, CEs = G[0], G[1], G[2], G[3]
    m1, q1, my1, qy1 = G[4], G[5], G[6], G[7]
    m2, q2, my2, qy2 = G[8], G[9], G[10], G[11]
    # per-core partials: sc2 [2,8] flattened at out[12:28]
    P = np.zeros((2, 8), dtype=np.float64)
    for o in outs:
        P += o[12:28].reshape(2, 8).astype(np.float64)
    ce_mean = CEs / max(Sw, EPS_W)

    def disco(row, Sr, m, q, my, qy):
        g_a, T1aa, g_b, T1bb, T1ab, S_ab = P[row, 0:6]
        s = 1.0 / max(Sr, EPS_W)
        dcov = s * s * S_ab - 2.0 * s**3 * T1ab + s**4 * g_a * g_b
        dvx = 2.0 * (s * q - (s * m) ** 2) - 2.0 * s**3 * T1aa + (s * s * g_a) ** 2
        dvy = 2.0 * (s * qy - (s * my) ** 2) - 2.0 * s**3 * T1bb + (s * s * g_b) ** 2
        ok = (dvx > EPS_VAR) and (dvy > EPS_VAR)
        if not ok:
            return 0.0
        return np.sqrt(np.abs(dcov) / np.sqrt(dvx * dvy))

    d1 = disco(0, S1, m1, q1, my1, qy1)
    d2 = disco(1, S2, m2, q2, my2, qy2)
    return np.float32(ce_mean + DISCO_LAMBDA * (d1 + d2))


# revision 49
# speedup vs baseline: 4010.0555x; 1.8634x over previous
"""DiSco weighted loss kernel for 8 trn2 NeuronCores.

Math: for symmetric a_ij=|x_i-x_j|, the weighted distance-correlation terms
collapse to  dcov = S_ab - 2*T1ab + g_a*g_b  with
  ar_i = sum_j w_j a_ij,  g_a = sum_i w_i ar_i,  T1ab = sum_i w_i ar_i br_i,
  S_ab = sum_ij w_i w_j a_ij b_ij,
and dvar_x = 2(q - m^2) - 2*T1aa + g_a^2 exactly (|.|^2 loses the abs).
Each core owns 512 i-rows (free axis) and scans all j (partition axis,
32 column-sets of its [128,32] f-major tiles); TensorE reduces over j via
bf16 matmuls accumulated in PSUM; the per-core scalar partials are summed
on the host (8x22 floats) to avoid a ~20us AllReduce latency floor.

Engine split: ACT builds the b=|dy| tiles (Abs activation, per-partition
bias) starting as soon as the masses land; DVE builds a=|dx| (4x
tensor_scalar + bitwise-and) and the a*b products (2x tensor_tensor); PE
accumulates five weighted row-sum streams in PSUM; GPSIMD (otherwise
idle) does all the prep (ny/masks/w12/w12row) and the CE + moment
statistics concurrently with the pairwise loop, so DVE's tail is just the
six fused phase-2 dot products.  The scr->xrow gather/broadcast DMAs are
issued from DVE's own queue to skip the SP queue's issue backlog.

Dispatch: the compiled executable (jax.jit of the bass_exec custom call,
sharded over the 8 cores) is cached in-process, so warm kernel() calls are
a single PJRT dispatch instead of a re-trace + re-lower every call.

`_build_program(reps)` emits the program `reps` times with per-iteration
semaphore-threshold offsets; reps>1 exists for in-NEFF repeat timing
(test.py measures the slope over reps to isolate device execution time
from the host/network dispatch floor).
"""

from contextlib import ExitStack

import numpy as np
import ml_dtypes

import concourse.bass as bass
from concourse import mybir

F32 = mybir.dt.float32
BF16 = mybir.dt.bfloat16
I32 = mybir.dt.int32
U16 = mybir.dt.uint16
AO = mybir.AluOpType
AF = mybir.ActivationFunctionType
AX = mybir.AxisListType

N, C, NCORES = 4096, 3, 8
M = N // NCORES  # 512 rows per core
NB = 32  # j-sets (columns of the [128,32] tiles)
LND = 6  # ACT loop group after which lnden is emitted

DISCO_LAMBDA = 0.1
EPS_W = 1e-8
EPS_VAR = 1e-10


def _build_program(reps=1, nb=NB):
    nc = bass.Bass()
    lg = nc.declare_dram_parameter("lg", [N, C], F32, isOutput=False)
    lgm = nc.declare_dram_parameter("lgm", [M, C], F32, isOutput=False)
    tg = nc.declare_dram_parameter("tg", [N], I32, isOutput=False)
    wf = nc.declare_dram_parameter("wf", [N], F32, isOutput=False)
    wm = nc.declare_dram_parameter("wm", [1, M], F32, isOutput=False)
    m1f = nc.declare_dram_parameter("m1f", [N], BF16, isOutput=False)
    m2f = nc.declare_dram_parameter("m2f", [N], BF16, isOutput=False)
    m1m = nc.declare_dram_parameter("m1m", [1, M], BF16, isOutput=False)
    m2m = nc.declare_dram_parameter("m2m", [1, M], BF16, isOutput=False)
    idn_d = nc.declare_dram_parameter("idn", [128, 128], BF16, isOutput=False)
    w4_d = nc.declare_dram_parameter("w4", [4, 512], BF16, isOutput=False)
    out = nc.declare_dram_parameter("out", [32], F32, isOutput=True)

    es = ExitStack()
    def sb(name, shp, dt):
        return es.enter_context(nc.sbuf_tensor(name, shp, dt))

    def ps(name, shp):
        return es.enter_context(nc.psum_tensor(name, shp, F32))

    lgt = sb("lgt", [128, 96], F32)
    lgtm = sb("lgtm", [128, 12], F32)
    tgt = sb("tgt", [128, 32], I32)
    wt = sb("wt", [128, 32], F32)
    m1t = sb("m1t", [128, 32], BF16)
    m2t = sb("m2t", [128, 32], BF16)
    wrow = sb("wrow", [2, M], F32)
    mrow = sb("mrow", [2, M], BF16)
    y1row = sb("y1row", [128, M], BF16)
    y2row = sb("y2row", [128, M], BF16)
    xrow = sb("xrow", [128, M], BF16)

    e = sb("e", [128, 96], F32)
    den = sb("den", [128, 32], F32)
    rec = sb("rec", [128, 32], F32)
    sc = sb("sc", [128, 32], F32)
    scbf = sb("scbf", [128, 32], BF16)
    sc_r = sb("sc_r", [128, 32], F32)  # bf16-rounded scores back in f32
    em = sb("em", [128, 12], F32)
    denm = sb("denm", [128, 4], F32)
    recm = sb("recm", [128, 4], F32)
    scm = sb("scm", [128, 4], F32)
    scmbf = sb("scmbf", [128, 4], BF16)
    ny1 = sb("ny1", [128, 32], F32)
    ny2 = sb("ny2", [128, 32], F32)
    msk1 = sb("msk1", [128, 32], F32)
    msk2 = sb("msk2", [128, 32], F32)
    w1 = sb("w1", [128, 32], F32)
    w2 = sb("w2", [128, 32], F32)
    w12 = sb("w12", [128, 64], BF16)
    m1ff = sb("m1ff", [128, 32], F32)
    m2ff = sb("m2ff", [128, 32], F32)
    sq = sb("sq", [128, 32], F32)
    pr = sb("pr", [128, 32], F32)
    pra = sb("pra", [128, 32], F32)
    tgtf = sb("tgtf", [128, 32], F32)
    lt = sb("lt", [128, 32], F32)
    lnden = sb("lnden", [128, 32], F32)
    ce = sb("ce", [128, 32], F32)
    G = sb("G", [128, 12], F32)
    Gsb = sb("Gsb", [1, 12], F32)
    ones_t = sb("ones_t", [128, 1], F32)
    idn = sb("idn_s", [128, 128], BF16)
    W4 = sb("w4_s", [4, 512], BF16)
    T4sb = sb("T4sb", [4, 128], BF16)

    DEPTH = 6  # loop-tile ring depth (PE may lag this many groups)
    d_ = [sb(f"d{i}", [128, 2 * M], BF16) for i in range(DEPTH)]
    a_ = [sb(f"a{i}", [128, 2 * M], BF16) for i in range(DEPTH)]
    b1_ = [sb(f"b1{i}", [128, 2 * M], BF16) for i in range(DEPTH)]
    b2_ = [sb(f"b2{i}", [128, 2 * M], BF16) for i in range(DEPTH)]
    ab1_ = [sb(f"ab1{i}", [128, 2 * M], BF16) for i in range(DEPTH)]
    ab2_ = [sb(f"ab2{i}", [128, 2 * M], BF16) for i in range(DEPTH)]

    mrow_w = sb("mrow_w", [2, M], F32)
    w12row = sb("w12row", [2, M], F32)
    arsb = sb("arsb", [2, M], F32)
    V = sb("V", [2, 2 * M], F32)
    t_ = sb("t_", [2, M], F32)
    t2_ = sb("t2_", [2, M], F32)
    t3_ = sb("t3_", [2, M], F32)
    sc2 = sb("sc2", [2, 8], F32)
    w12row_bf = sb("w12row_bf", [2, M], BF16)
    st_b2 = sb("st_b2", [2, M], F32)
    st_p2 = sb("st_p2", [2, M], F32)

    ps_a = ps("ps_a", [2, M])
    ps_b1 = ps("ps_b1", [2, M])
    ps_b2 = ps("ps_b2", [2, M])
    ps_p1 = ps("ps_p1", [2, M])
    ps_p2 = ps("ps_p2", [2, M])
    ps_g = ps("ps_g", [1, 12])
    ps_t = es.enter_context(nc.psum_tensor("ps_t", [4, 128], BF16))  # transposed own-scores
    ps_x = ps("ps_x", [128, M])   # broadcast own-scores

    dmA = es.enter_context(nc.semaphore("dmA"))    # lgt, lgtm
    dmB = es.enter_context(nc.semaphore("dmB"))    # m1t, m2t, wt
    dmC = es.enter_context(nc.semaphore("dmC"))    # y1row, y2row (Pool queue)
    dmD = es.enter_context(nc.semaphore("dmD"))    # tgt, wrow, mrow x2
    sa = es.enter_context(nc.semaphore("sa"))      # ACT progress
    svp = es.enter_context(nc.semaphore("svp"))    # DVE score-chain progress
    sv = es.enter_context(nc.semaphore("sv"))      # DVE loop progress
    sp = es.enter_context(nc.semaphore("sp"))      # PE progress
    sa2 = es.enter_context(nc.semaphore("sa2"))    # ACT V copies
    spool = es.enter_context(nc.semaphore("spool"))  # Pool progress
    spb = es.enter_context(nc.semaphore("spb"))      # PE b-part progress
    spx = es.enter_context(nc.semaphore("spx"))      # PE score-broadcast progress
    dmV = es.enter_context(nc.semaphore("dmV"))      # V row-1 DMA moves
    dmE = es.enter_context(nc.semaphore("dmE"))      # idn/w4 constant loads
    block = es.enter_context(nc.Block())

    # per-iteration semaphore increments
    DMA_IT, DMB_IT, DMC_IT, DMD_IT = 32, 48, 32, 64
    SPX_IT = 2
    SA_IT = 3 + 4 * (nb // 2)
    SVP_IT = 6
    SV_IT = 2 * (nb // 2) + 200
    SP_IT = 1 + nb // 2
    SA2_IT = 4
    SPOOL_IT = 3

    def sa_val(K, v):
        # sa value after the v-th activation of loop group K (v=1..4);
        # em, e, lnden precede the loop
        return 3 + 4 * K + v

    for r in range(reps):
        DMAB, DMBB, DMCB, DMDB = DMA_IT * r, DMB_IT * r, DMC_IT * r, DMD_IT * r
        SPXB = SPX_IT * r
        DMVB = 64 * r
        SAB = SA_IT * r
        SVPB = SVP_IT * r
        SVB = SV_IT * r
        SPB = SP_IT * r
        SA2B = SA2_IT * r
        SPOOLB = SPOOL_IT * r
        SPBB = (nb // 2) * r

        @block.sync
        def _(sync):
            sync.dma_start(out=lgtm[:].rearrange("p (q c) -> p q c", c=3), in_=lgm[:].rearrange("(q p) c -> p q c", q=4)).then_inc(dmA, 16)
            sync.dma_start(out=lgt[:], in_=lg[:].rearrange("(p f) c -> p (f c)", f=32)).then_inc(dmA, 16)
            if r == 0:
                sync.dma_start(out=idn[:], in_=idn_d[:]).then_inc(dmE, 16)
                sync.dma_start(out=W4[:], in_=w4_d[:]).then_inc(dmE, 16)
            sync.dma_start(out=m1t[:], in_=m1f[:].rearrange("(p f) -> p f", f=32)).then_inc(dmB, 16)
            sync.dma_start(out=m2t[:], in_=m2f[:].rearrange("(p f) -> p f", f=32)).then_inc(dmB, 16)
            sync.dma_start(out=wt[:], in_=wf[:].rearrange("(p f) -> p f", f=32)).then_inc(dmB, 16)
            sync.dma_start(out=y1row[:], in_=m1m[:].broadcast_to([128, M])).then_inc(dmC, 16)
            sync.dma_start(out=y2row[:], in_=m2m[:].broadcast_to([128, M])).then_inc(dmC, 16)
            sync.dma_start(out=tgt[:], in_=tg[:].rearrange("(p f) -> p f", f=32)).then_inc(dmD, 16)
            sync.dma_start(out=wrow[:], in_=wm[:].broadcast_to([2, M])).then_inc(dmD, 16)
            sync.dma_start(out=mrow[0:1, :], in_=m1m[:]).then_inc(dmD, 16)
            sync.dma_start(out=mrow[1:2, :], in_=m2m[:]).then_inc(dmD, 16)
            # V row-1 moves
            sync.wait_ge(sa2, SA2B + 2)
            sync.dma_start(out=V[1:2, 0:M], in_=st_b2[1:2, :]).then_inc(dmV, 16)
            sync.wait_ge(sa2, SA2B + 4)
            sync.dma_start(out=V[1:2, M : 2 * M], in_=st_p2[1:2, :]).then_inc(dmV, 16)
            # outputs
            sync.wait_ge(sv, SVB + SV_IT)
            sync.dma_start(out=out[0:12], in_=Gsb[:]).then_inc(dmV, 16)
            sync.dma_start(out=out[12:28], in_=sc2[:]).then_inc(dmV, 16)

        @block.gpsimd
        def _(gp):
            gp.memset(ones_t[:], 1.0)

        @block.scalar
        def _(scalar):
            scalar.wait_ge(dmA, DMAB + 16)
            scalar.activation(em[:], lgtm[:], AF.Exp).then_inc(sa, 1)
            scalar.wait_ge(dmA, DMAB + 32)
            scalar.activation(e[:], lgt[:], AF.Exp).then_inc(sa, 1)
            scalar.wait_ge(svp, SVPB + 4)  # den
            scalar.activation(lnden[:], den[:], AF.Ln).then_inc(sa, 1)
            # b tiles: start as soon as ny1/ny2 + y rows are in
            scalar.wait_ge(svp, SVPB + 1)
            scalar.wait_ge(dmC, DMCB + 32)
            for K in range(nb // 2):
                s = K % DEPTH
                spw = SPBB + K - DEPTH + 1
                if spw > 0:
                    scalar.wait_ge(spb, spw)
                k0, k1 = 2 * K, 2 * K + 1
                scalar.activation(b1_[s][:, 0:M], y1row[:], AF.Abs, bias=ny1[:, k0 : k0 + 1]).then_inc(sa, 1)
                scalar.activation(b1_[s][:, M : 2 * M], y1row[:], AF.Abs, bias=ny1[:, k1 : k1 + 1]).then_inc(sa, 1)
                scalar.activation(b2_[s][:, 0:M], y2row[:], AF.Abs, bias=ny2[:, k0 : k0 + 1]).then_inc(sa, 1)
                scalar.activation(b2_[s][:, M : 2 * M], y2row[:], AF.Abs, bias=ny2[:, k1 : k1 + 1]).then_inc(sa, 1)
            # V assembly: partition-0 copies; row-1 halves moved by SP DMAs
            scalar.wait_ge(spb, SPBB + nb // 2)
            scalar.activation(V[0:1, 0:M], ps_b1[0:1, :], AF.Copy).then_inc(sa2, 1)
            scalar.activation(st_b2[:], ps_b2[:], AF.Copy).then_inc(sa2, 1)
            scalar.wait_ge(sp, SPB + nb // 2)
            scalar.activation(V[0:1, M : 2 * M], ps_p1[0:1, :], AF.Copy).then_inc(sa2, 1)
            scalar.activation(st_p2[:], ps_p2[:], AF.Copy).then_inc(sa2, 1)

        @block.vector
        def _(vector):
            # prep: negated masses, masked weights, bf16 weight pairs
            vector.wait_ge(dmB, DMBB + 48)
            vector.tensor_scalar(ny1[:], m1t[:], -1.0, None, AO.mult)
            vector.tensor_scalar(ny2[:], m2t[:], -1.0, None, AO.mult)
            vector.drain().then_inc(svp, 1)
            vector.tensor_scalar(msk1[:], m1t[:], 0.0, None, AO.is_gt)
            vector.tensor_scalar(msk2[:], m2t[:], 0.0, None, AO.is_gt)
            vector.tensor_tensor(w1[:], wt[:], msk1[:], AO.mult)
            vector.tensor_tensor(w2[:], wt[:], msk2[:], AO.mult)
            vector.drain()
            vector.tensor_copy(w12[:, 0:64:2], w1[:])
            vector.tensor_copy(w12[:, 1:64:2], w2[:])
            vector.drain().then_inc(svp, 1)
            # own-row softmax scores (gate the transpose/broadcast chain)
            vector.wait_ge(sa, SAB + 1)
            vector.tensor_reduce(denm[:], em[:].rearrange("p (f c) -> p f c", c=3), AX.X, AO.add)
            vector.drain()
            vector.reciprocal(recm[:], denm[:])
            vector.drain()
            vector.tensor_tensor(scm[:], em[:, 0:12:3], recm[:], AO.mult)
            vector.drain()
            vector.tensor_copy(scmbf[:], scm[:]).then_inc(svp, 1)
            # full-N scores
            vector.wait_ge(sa, SAB + 2)
            vector.tensor_reduce(den[:], e[:].rearrange("p (f c) -> p f c", c=3), AX.X, AO.add)
            vector.drain()
            vector.reciprocal(rec[:], den[:])
            vector.drain()
            vector.tensor_tensor(sc[:], e[:, 0:96:3], rec[:], AO.mult)
            vector.drain()
            vector.tensor_copy(scbf[:], sc[:])
            vector.drain()
            vector.tensor_copy(sc_r[:], scbf[:]).then_inc(svp, 1)
            # own-score broadcast: PSUM transpose -> SBUF -> PSUM bcast -> xrow
            vector.wait_ge(spx, SPXB + 1)
            vector.tensor_copy(T4sb[:], ps_t[:]).then_inc(svp, 1)
            vector.wait_ge(spx, SPXB + 2)
            vector.tensor_copy(xrow[:], ps_x[:])
            # ---- pairwise loop, with CE/G statistics interleaved into
            # the per-iteration slack (DVE is ~0.5us/group lighter than ACT) ----
            def stats(K):
                if K == 1:
                    vector.wait_ge(dmD, DMDB + 64)
                    vector.tensor_copy(tgtf[:], tgt[:])
                    vector.tensor_copy(m1ff[:], m1t[:])
                elif K == 2:
                    vector.tensor_copy(m2ff[:], m2t[:])
                    vector.tensor_tensor(sq[:], sc_r[:], sc_r[:], AO.mult)
                elif K == 3:
                    vector.scalar_tensor_tensor(lt[:], tgtf[:], 0.0, lgt[:, 0:96:3], AO.is_equal, AO.mult)
                    vector.scalar_tensor_tensor(pr[:], tgtf[:], 1.0, lgt[:, 1:96:3], AO.is_equal, AO.mult)
                elif K == 4:
                    vector.tensor_tensor(lt[:], lt[:], pr[:], AO.add)
                    vector.scalar_tensor_tensor(pr[:], tgtf[:], 2.0, lgt[:, 2:96:3], AO.is_equal, AO.mult)
                elif K == 5:
                    vector.tensor_tensor(lt[:], lt[:], pr[:], AO.add)
                    vector.tensor_scalar(mrow_w[:], mrow[:], 0.0, None, AO.is_gt)
                elif K == 6:
                    vector.tensor_tensor(w12row[:], wrow[:], mrow_w[:], AO.mult)
                elif K == 7:
                    vector.tensor_copy(w12row_bf[:], w12row[:])
                elif K == 8:
                    vector.tensor_copy(w12row[:], w12row_bf[:])
                    vector.wait_ge(sa, SAB + 3)  # lnden
                    vector.tensor_tensor(ce[:], lnden[:], lt[:], AO.subtract)
                elif K == 9:
                    vector.scalar_tensor_tensor(pr[:], wt[:], 0.0, wt[:], AO.is_ge, AO.mult, accum_out=G[:, 0:1])
                    vector.scalar_tensor_tensor(pr[:], wt[:], 1.0, msk1[:], AO.mult, AO.mult, accum_out=G[:, 1:2])
                elif K == 10:
                    vector.scalar_tensor_tensor(pr[:], wt[:], 1.0, msk2[:], AO.mult, AO.mult, accum_out=G[:, 2:3])
                    vector.scalar_tensor_tensor(pr[:], wt[:], 1.0, ce[:], AO.mult, AO.mult, accum_out=G[:, 3:4])
                elif K == 11:
                    vector.scalar_tensor_tensor(pr[:], w1[:], 1.0, sc_r[:], AO.mult, AO.mult, accum_out=G[:, 4:5])
                    vector.scalar_tensor_tensor(pr[:], w1[:], 1.0, sq[:], AO.mult, AO.mult, accum_out=G[:, 5:6])
                elif K == 12:
                    vector.scalar_tensor_tensor(pra[:], w1[:], 1.0, m1ff[:], AO.mult, AO.mult, accum_out=G[:, 6:7])
                    vector.scalar_tensor_tensor(pr[:], w2[:], 1.0, sc_r[:], AO.mult, AO.mult, accum_out=G[:, 8:9])
                elif K == 13:
                    vector.scalar_tensor_tensor(pr[:], pra[:], 1.0, m1ff[:], AO.mult, AO.mult, accum_out=G[:, 7:8])
                    vector.scalar_tensor_tensor(pr[:], w2[:], 1.0, sq[:], AO.mult, AO.mult, accum_out=G[:, 9:10])
                elif K == 14:
                    vector.scalar_tensor_tensor(pra[:], w2[:], 1.0, m2ff[:], AO.mult, AO.mult, accum_out=G[:, 10:11])
                elif K == 15:
                    vector.scalar_tensor_tensor(pr[:], pra[:], 1.0, m2ff[:], AO.mult, AO.mult, accum_out=G[:, 11:12])
                    vector.drain().then_inc(svp, 1)

            for K in range(nb // 2):
                s = K % DEPTH
                spw = SPB + K - DEPTH + 1 if K >= DEPTH else SPB + K - DEPTH
                if spw > 0:
                    vector.wait_ge(sp, spw)
                if nb == NB:
                    stats(K)
                k0, k1 = 2 * K, 2 * K + 1
                vector.tensor_scalar(d_[s][:, 0:M], xrow[:], sc_r[:, k0 : k0 + 1], None, AO.subtract)
                vector.tensor_scalar(d_[s][:, M : 2 * M], xrow[:], sc_r[:, k1 : k1 + 1], None, AO.subtract)
                vector.tensor_scalar(
                    a_[s][:].bitcast(U16), d_[s][:].bitcast(U16), 0x7FFF, None, AO.bitwise_and
                ).then_inc(sv, 1)
                vector.wait_ge(sa, SAB + sa_val(K, 2))
                vector.tensor_tensor(ab1_[s][:], a_[s][:], b1_[s][:], AO.mult)
                vector.wait_ge(sa, SAB + sa_val(K, 4))
                vector.tensor_tensor(ab2_[s][:], a_[s][:], b2_[s][:], AO.mult).then_inc(sv, 1)
            if nb != NB:
                # timing variants: emit all stats after the loop
                for K in range(16):
                    stats(K)
            # ---- phase 2: fused dot products via stt-accum ----
            vector.wait_ge(sp, SPB + nb // 2)
            vector.tensor_copy(arsb[:], ps_a[:])
            vector.drain()
            vector.scalar_tensor_tensor(t_[:], arsb[:], 1.0, w12row[:], AO.mult, AO.mult, accum_out=sc2[:, 0:1])   # g_a
            vector.scalar_tensor_tensor(t3_[:], t_[:], 1.0, arsb[:], AO.mult, AO.mult, accum_out=sc2[:, 1:2])      # T1aa
            vector.wait_ge(sa2, SA2B + 1)
            vector.wait_ge(dmV, DMVB + 16)
            vector.scalar_tensor_tensor(t3_[:], t_[:], 1.0, V[:, 0:M], AO.mult, AO.mult, accum_out=sc2[:, 4:5])    # T1ab
            vector.scalar_tensor_tensor(t2_[:], V[:, 0:M], 1.0, w12row[:], AO.mult, AO.mult, accum_out=sc2[:, 2:3])  # g_b
            vector.scalar_tensor_tensor(t3_[:], t2_[:], 1.0, V[:, 0:M], AO.mult, AO.mult, accum_out=sc2[:, 3:4])   # T1bb
            vector.wait_ge(sa2, SA2B + 3)
            vector.wait_ge(dmV, DMVB + 32)
            vector.scalar_tensor_tensor(t3_[:], V[:, M : 2 * M], 1.0, w12row[:], AO.mult, AO.mult, accum_out=sc2[:, 5:6])  # S_ab
            vector.wait_ge(sp, SPB + nb // 2 + 1)
            vector.tensor_copy(Gsb[:], ps_g[:]).then_inc(sv, 200)

        @block.tensor
        def _(tensor):
            def bpart(J):
                s = J % DEPTH
                st = J == 0
                last = J == nb // 2 - 1
                k0, k1 = 2 * J, 2 * J + 1
                lw0 = w12[:, 2 * k0 : 2 * k0 + 2]
                lw1 = w12[:, 2 * k1 : 2 * k1 + 2]
                tensor.wait_ge(sa, SAB + sa_val(J, 4))
                tensor.matmul(ps_b1[:], lw0, b1_[s][:, 0:M], start=st, stop=False)
                tensor.matmul(ps_b2[:], lw0, b2_[s][:, 0:M], start=st, stop=False)
                tensor.matmul(ps_b1[:], lw1, b1_[s][:, M : 2 * M], start=False, stop=last)
                tensor.matmul(ps_b2[:], lw1, b2_[s][:, M : 2 * M], start=False, stop=last).then_inc(spb, 1)

            def abpart(K):
                s = K % DEPTH
                st = K == 0
                last = K == nb // 2 - 1
                k0, k1 = 2 * K, 2 * K + 1
                lw0 = w12[:, 2 * k0 : 2 * k0 + 2]
                lw1 = w12[:, 2 * k1 : 2 * k1 + 2]
                tensor.wait_ge(sv, SVB + 2 * K + 2)
                tensor.matmul(ps_a[:], lw0, a_[s][:, 0:M], start=st, stop=False)
                tensor.matmul(ps_p1[:], lw0, ab1_[s][:, 0:M], start=st, stop=False)
                tensor.matmul(ps_p2[:], lw0, ab2_[s][:, 0:M], start=st, stop=False)
                tensor.matmul(ps_a[:], lw1, a_[s][:, M : 2 * M], start=False, stop=last)
                tensor.matmul(ps_p1[:], lw1, ab1_[s][:, M : 2 * M], start=False, stop=last)
                tensor.matmul(ps_p2[:], lw1, ab2_[s][:, M : 2 * M], start=False, stop=last).then_inc(sp, 1)

            # own-score transpose + broadcast (idn/W4 are constant inputs)
            tensor.wait_ge(dmE, 32)
            tensor.wait_ge(svp, SVPB + 3)
            tensor.transpose(ps_t[:], scmbf[:], idn[:]).then_inc(spx, 1)
            tensor.wait_ge(svp, SVPB + 5)
            for c in range(4):
                mm = tensor.matmul(ps_x[:, 128 * c : 128 * (c + 1)], W4[:, 128 * c : 128 * (c + 1)], T4sb[:], start=True, stop=True)
            mm.then_inc(spx, 1)
            tensor.wait_ge(svp, SVPB + 2)  # w12 ready
            bpart(0)
            for K in range(nb // 2):
                if K + 1 < nb // 2:
                    bpart(K + 1)
                abpart(K)
            tensor.wait_ge(svp, SVPB + 6)  # G ready
            tensor.matmul(ps_g[:], ones_t[:], G[:], start=True, stop=True).then_inc(sp, 1)

    return nc, es


_NC_CACHE = {}


def _get_runner(reps=1, nb=NB):
    """Build (once) and cache the program + AOT-jitted single-dispatch callable."""
    key = ("runner", reps, nb)
    if key in _NC_CACHE:
        return _NC_CACHE[key]

    import jax
    from jax.sharding import Mesh, PartitionSpec
    from jax.experimental.shard_map import shard_map
    import concourse.bass2jax as b2j

    nc, _es = _build_program(reps, nb)
    b2j.install_neuronx_cc_hook()

    partition_name = nc.partition_id_tensor.name if nc.partition_id_tensor else None
    in_names, out_names, out_avals, out_shapes = [], [], [], []
    for alloc in nc.m.functions[0].allocations:
        if not isinstance(alloc, mybir.MemoryLocationSet):
            continue
        name = alloc.memorylocations[0].name
        if alloc.kind == "ExternalInput":
            if name != partition_name:
                in_names.append(name)
        elif alloc.kind == "ExternalOutput":
            out_names.append(name)
            shape = tuple(alloc.tensor_shape)
            dtype = mybir.dt.np(alloc.dtype)
            out_avals.append(jax.core.ShapedArray(shape, dtype))
            out_shapes.append((shape, dtype))
    n_params = len(in_names)
    n_outs = len(out_names)
    in_names_all = list(in_names) + list(out_names)
    if partition_name is not None:
        in_names_all.append(partition_name)

    def _body(*args):
        operands = list(args)
        if partition_name is not None:
            operands.append(b2j.partition_id_tensor())
        outs = b2j._bass_exec_p.bind(
            *operands,
            out_avals=tuple(out_avals),
            in_names=tuple(in_names_all),
            out_names=tuple(out_names),
            lowering_input_output_aliases=(),
            sim_require_finite=True,
            sim_require_nnan=True,
            nc=nc,
        )
        return tuple(outs)

    devices = jax.devices()[:NCORES]
    mesh = Mesh(np.asarray(devices), ("core",))
    in_specs = (PartitionSpec("core"),) * (n_params + n_outs)
    out_specs = (PartitionSpec("core"),) * n_outs
    jitted = jax.jit(
        shard_map(_body, mesh=mesh, in_specs=in_specs, out_specs=out_specs, check_rep=False),
        keep_unused=True,
    )

    def run(in_maps):
        concat_in = [
            np.concatenate([np.asarray(in_maps[c][name]) for c in range(NCORES)], axis=0)
            for name in in_names
        ]
        zeros = [np.zeros((NCORES * s[0], *s[1:]), d) for s, d in out_shapes]
        outs = jitted(*concat_in, *zeros)
        jax.block_until_ready(outs)
        return [
            {
                name: np.asarray(outs[i]).reshape(NCORES, *out_shapes[i][0])[c]
                for i, name in enumerate(out_names)
            }
            for c in range(NCORES)
        ]

    _NC_CACHE[key] = run
    return run


_IDN = np.eye(128, dtype=ml_dtypes.bfloat16)
_W4 = np.zeros((4, 512), dtype=ml_dtypes.bfloat16)
for _c in range(4):
    _W4[_c, 128 * _c : 128 * (_c + 1)] = 1


def _make_in_maps(logits, target_i, weight, m1b, m2b):
    in_maps = []
    for c in range(NCORES):
        sl = slice(c * M, (c + 1) * M)
        in_maps.append(
            {
                "idn": _IDN,
                "w4": _W4,
                "lg": logits,
                "lgm": np.ascontiguousarray(logits[sl]),
                "tg": target_i,
                "wf": weight,
                "wm": weight[sl].reshape(1, M),
                "m1f": m1b,
                "m2f": m2b,
                "m1m": m1b[sl].reshape(1, M),
                "m2m": m2b[sl].reshape(1, M),
            }
        )
    return in_maps


def kernel(logits, target, weight, mass1, mass2):
    logits = np.asarray(logits, dtype=np.float32)
    target_i = np.asarray(target).astype(np.int32)
    weight = np.asarray(weight, dtype=np.float32)
    mass1 = np.asarray(mass1, dtype=np.float32)
    mass2 = np.asarray(mass2, dtype=np.float32)
    m1b = mass1.astype(ml_dtypes.bfloat16)
    m2b = mass2.astype(ml_dtypes.bfloat16)

    run = _get_runner(reps=1)
    res = run(_make_in_maps(logits, target_i, weight, m1b, m2b))
    outs = [r["out"] for r in res]
    return _combine(outs)


def _combine(outs):
    G = np.asarray(outs[0][0:12], dtype=np.float64)
    Sw, S1, S2, CEs = G[0], G[1], G[2], G[3]
    m1, q1, my1, qy1 = G[4], G[5], G[6], G[7]
    m2, q2, my2, qy2 = G[8], G[9], G[10], G[11]
    # per-core partials: sc2 [2,8] flattened at out[12:28]
    P = np.zeros((2, 8), dtype=np.float64)
    for o in outs:
        P += o[12:28].reshape(2, 8).astype(np.float64)
    ce_mean = CEs / max(Sw, EPS_W)

    def disco(row, Sr, m, q, my, qy):
        g_a, T1aa, g_b, T1bb, T1ab, S_ab = P[row, 0:6]
        s = 1.0 / max(Sr, EPS_W)
        dcov = s * s * S_ab - 2.0 * s**3 * T1ab + s**4 * g_a * g_b
        dvx = 2.0 * (s * q - (s * m) ** 2) - 2.0 * s**3 * T1aa + (s * s * g_a) ** 2
        dvy = 2.0 * (s * qy - (s * my) ** 2) - 2.0 * s**3 * T1bb + (s * s * g_b) ** 2
        ok = (dvx > EPS_VAR) and (dvy > EPS_VAR)
        if not ok:
            return 0.0
        return np.sqrt(np.abs(dcov) / np.sqrt(dvx * dvy))

    d1 = disco(0, S1, m1, q1, my1, qy1)
    d2 = disco(1, S2, m2, q2, my2, qy2)
    return np.float32(ce_mean + DISCO_LAMBDA * (d1 + d2))


# revision 50
# speedup vs baseline: 4679.6470x; 1.1670x over previous
"""DiSco weighted loss kernel for 8 trn2 NeuronCores.

Math: for symmetric a_ij=|x_i-x_j|, the weighted distance-correlation terms
collapse to  dcov = S_ab - 2*T1ab + g_a*g_b  with
  ar_i = sum_j w_j a_ij,  g_a = sum_i w_i ar_i,  T1ab = sum_i w_i ar_i br_i,
  S_ab = sum_ij w_i w_j a_ij b_ij,
and dvar_x = 2(q - m^2) - 2*T1aa + g_a^2 exactly (|.|^2 loses the abs).
Each core owns 512 i-rows (free axis) and scans all j (partition axis,
32 column-sets of its [128,32] f-major tiles); TensorE reduces over j via
bf16 matmuls accumulated in PSUM; the per-core scalar partials are summed
on the host (8x22 floats) to avoid a ~20us AllReduce latency floor.

Engine split: ACT builds the b=|dy| tiles (Abs activation, per-partition
bias) starting as soon as the masses land; DVE builds a=|dx| (4x
tensor_scalar + bitwise-and) and the a*b products (2x tensor_tensor); PE
accumulates five weighted row-sum streams in PSUM; GPSIMD (otherwise
idle) does all the prep (ny/masks/w12/w12row) and the CE + moment
statistics concurrently with the pairwise loop, so DVE's tail is just the
six fused phase-2 dot products.  The scr->xrow gather/broadcast DMAs are
issued from DVE's own queue to skip the SP queue's issue backlog.

Dispatch: the compiled executable (jax.jit of the bass_exec custom call,
sharded over the 8 cores) is cached in-process, so warm kernel() calls are
a single PJRT dispatch instead of a re-trace + re-lower every call.

`_build_program(reps)` emits the program `reps` times with per-iteration
semaphore-threshold offsets; reps>1 exists for in-NEFF repeat timing
(test.py measures the slope over reps to isolate device execution time
from the host/network dispatch floor).
"""

from contextlib import ExitStack

import numpy as np
import ml_dtypes

import concourse.bass as bass
from concourse import mybir

F32 = mybir.dt.float32
BF16 = mybir.dt.bfloat16
I32 = mybir.dt.int32
U16 = mybir.dt.uint16
AO = mybir.AluOpType
AF = mybir.ActivationFunctionType
AX = mybir.AxisListType

N, C, NCORES = 4096, 3, 8
M = N // NCORES  # 512 rows per core
NB = 32  # j-sets (columns of the [128,32] tiles)
LND = 6  # ACT loop group after which lnden is emitted

DISCO_LAMBDA = 0.1
EPS_W = 1e-8
EPS_VAR = 1e-10


def _build_program(reps=1, nb=NB):
    nc = bass.Bass()
    lg = nc.declare_dram_parameter("lg", [N, C], F32, isOutput=False)
    lgm = nc.declare_dram_parameter("lgm", [M, C], F32, isOutput=False)
    tg = nc.declare_dram_parameter("tg", [N], I32, isOutput=False)
    wf = nc.declare_dram_parameter("wf", [N], F32, isOutput=False)
    wm = nc.declare_dram_parameter("wm", [1, M], F32, isOutput=False)
    m1f = nc.declare_dram_parameter("m1f", [N], BF16, isOutput=False)
    m2f = nc.declare_dram_parameter("m2f", [N], BF16, isOutput=False)
    m1m = nc.declare_dram_parameter("m1m", [1, M], BF16, isOutput=False)
    m2m = nc.declare_dram_parameter("m2m", [1, M], BF16, isOutput=False)
    idn_d = nc.declare_dram_parameter("idn", [128, 128], BF16, isOutput=False)
    w4_d = nc.declare_dram_parameter("w4", [4, 512], BF16, isOutput=False)
    out = nc.declare_dram_parameter("out", [32], F32, isOutput=True)

    es = ExitStack()
    def sb(name, shp, dt):
        return es.enter_context(nc.sbuf_tensor(name, shp, dt))

    def ps(name, shp):
        return es.enter_context(nc.psum_tensor(name, shp, F32))

    lgt = sb("lgt", [128, 96], F32)
    lgtm = sb("lgtm", [128, 12], F32)
    tgt = sb("tgt", [128, 32], I32)
    wt = sb("wt", [128, 32], F32)
    m1t = sb("m1t", [128, 32], BF16)
    m2t = sb("m2t", [128, 32], BF16)
    wrow = sb("wrow", [2, M], F32)
    mrow = sb("mrow", [2, M], BF16)
    y1row = sb("y1row", [128, M], BF16)
    y2row = sb("y2row", [128, M], BF16)
    xrow = sb("xrow", [128, M], BF16)

    e = sb("e", [128, 96], F32)
    den = sb("den", [128, 32], F32)
    rec = sb("rec", [128, 32], F32)
    sc = sb("sc", [128, 32], F32)
    scbf = sb("scbf", [128, 32], BF16)
    sc_r = sb("sc_r", [128, 32], F32)  # bf16-rounded scores back in f32
    em = sb("em", [128, 12], F32)
    denm = sb("denm", [128, 4], F32)
    recm = sb("recm", [128, 4], F32)
    scm = sb("scm", [128, 4], F32)
    scmbf = sb("scmbf", [128, 4], BF16)
    ny1 = sb("ny1", [128, 32], F32)
    ny2 = sb("ny2", [128, 32], F32)
    msk1 = sb("msk1", [128, 32], F32)
    msk2 = sb("msk2", [128, 32], F32)
    w1 = sb("w1", [128, 32], F32)
    w2 = sb("w2", [128, 32], F32)
    w12 = sb("w12", [128, 64], BF16)
    m1ff = sb("m1ff", [128, 32], F32)
    m2ff = sb("m2ff", [128, 32], F32)
    sq = sb("sq", [128, 32], F32)
    pr = sb("pr", [128, 32], F32)
    pra = sb("pra", [128, 32], F32)
    tgtf = sb("tgtf", [128, 32], F32)
    lt = sb("lt", [128, 32], F32)
    lnden = sb("lnden", [128, 32], F32)
    ce = sb("ce", [128, 32], F32)
    G = sb("G", [128, 12], F32)
    Gsb = sb("Gsb", [1, 12], F32)
    ones_t = sb("ones_t", [128, 1], F32)
    idn = sb("idn_s", [128, 128], BF16)
    W4 = sb("w4_s", [4, 512], BF16)
    T4sb = sb("T4sb", [4, 128], BF16)

    DEPTH = 6  # loop-tile ring depth (PE may lag this many groups)
    d_ = [sb(f"d{i}", [128, 2 * M], BF16) for i in range(DEPTH)]
    a_ = [sb(f"a{i}", [128, 2 * M], BF16) for i in range(DEPTH)]
    b1_ = [sb(f"b1{i}", [128, 2 * M], BF16) for i in range(DEPTH)]
    b2_ = [sb(f"b2{i}", [128, 2 * M], BF16) for i in range(DEPTH)]
    ab1_ = [sb(f"ab1{i}", [128, 2 * M], BF16) for i in range(DEPTH)]
    ab2_ = [sb(f"ab2{i}", [128, 2 * M], BF16) for i in range(DEPTH)]

    mrow_w = sb("mrow_w", [2, M], F32)
    w12row = sb("w12row", [2, M], F32)
    arsb = sb("arsb", [2, M], BF16)
    V = sb("V", [2, 2 * M], BF16)
    t_ = sb("t_", [2, M], BF16)
    t2_ = sb("t2_", [2, M], BF16)
    t3_ = sb("t3_", [2, M], BF16)
    sc2 = sb("sc2", [2, 8], F32)
    w12row_bf = sb("w12row_bf", [2, M], BF16)
    st_b2 = sb("st_b2", [2, M], BF16)
    st_p2 = sb("st_p2", [2, M], BF16)

    ps_a = ps("ps_a", [2, M])
    ps_b1 = ps("ps_b1", [2, M])
    ps_b2 = ps("ps_b2", [2, M])
    ps_p1 = ps("ps_p1", [2, M])
    ps_p2 = ps("ps_p2", [2, M])
    ps_g = ps("ps_g", [1, 12])
    ps_t = es.enter_context(nc.psum_tensor("ps_t", [4, 128], BF16))  # transposed own-scores
    ps_x = ps("ps_x", [128, M])   # broadcast own-scores

    dmA = es.enter_context(nc.semaphore("dmA"))    # lgt, lgtm
    dmB = es.enter_context(nc.semaphore("dmB"))    # m1t, m2t, wt
    dmC = es.enter_context(nc.semaphore("dmC"))    # y1row, y2row (Pool queue)
    dmD = es.enter_context(nc.semaphore("dmD"))    # tgt, wrow, mrow x2
    sa = es.enter_context(nc.semaphore("sa"))      # ACT progress
    svp = es.enter_context(nc.semaphore("svp"))    # DVE score-chain progress
    sv = es.enter_context(nc.semaphore("sv"))      # DVE loop progress
    sp = es.enter_context(nc.semaphore("sp"))      # PE progress
    sa2 = es.enter_context(nc.semaphore("sa2"))    # ACT V copies
    spool = es.enter_context(nc.semaphore("spool"))  # Pool progress
    spb = es.enter_context(nc.semaphore("spb"))      # PE b-part progress
    spx = es.enter_context(nc.semaphore("spx"))      # PE score-broadcast progress
    dmV = es.enter_context(nc.semaphore("dmV"))      # V row-1 DMA moves
    dmE = es.enter_context(nc.semaphore("dmE"))      # idn/w4 constant loads
    block = es.enter_context(nc.Block())

    # per-iteration semaphore increments
    DMA_IT, DMB_IT, DMC_IT, DMD_IT = 32, 48, 32, 64
    SPX_IT = 2
    SA_IT = 3 + 4 * (nb // 2)
    SVP_IT = 6
    SV_IT = 2 * (nb // 2) + 200
    SP_IT = 1 + nb // 2
    SA2_IT = 4
    SPOOL_IT = 3

    def sa_val(K, v):
        # sa value after the v-th activation of loop group K (v=1..4);
        # em, e, lnden precede the loop
        return 3 + 4 * K + v

    for r in range(reps):
        DMAB, DMBB, DMCB, DMDB = DMA_IT * r, DMB_IT * r, DMC_IT * r, DMD_IT * r
        SPXB = SPX_IT * r
        DMVB = 64 * r
        SAB = SA_IT * r
        SVPB = SVP_IT * r
        SVB = SV_IT * r
        SPB = SP_IT * r
        SA2B = SA2_IT * r
        SPOOLB = SPOOL_IT * r
        SPBB = (nb // 2) * r

        @block.sync
        def _(sync):
            sync.dma_start(out=lgtm[:].rearrange("p (q c) -> p q c", c=3), in_=lgm[:].rearrange("(q p) c -> p q c", q=4)).then_inc(dmA, 16)
            sync.dma_start(out=lgt[:], in_=lg[:].rearrange("(p f) c -> p (f c)", f=32)).then_inc(dmA, 16)
            if r == 0:
                sync.dma_start(out=idn[:], in_=idn_d[:]).then_inc(dmE, 16)
                sync.dma_start(out=W4[:], in_=w4_d[:]).then_inc(dmE, 16)
            sync.dma_start(out=m1t[:], in_=m1f[:].rearrange("(p f) -> p f", f=32)).then_inc(dmB, 16)
            sync.dma_start(out=m2t[:], in_=m2f[:].rearrange("(p f) -> p f", f=32)).then_inc(dmB, 16)
            sync.dma_start(out=wt[:], in_=wf[:].rearrange("(p f) -> p f", f=32)).then_inc(dmB, 16)
            sync.dma_start(out=y1row[:], in_=m1m[:].broadcast_to([128, M])).then_inc(dmC, 16)
            sync.dma_start(out=y2row[:], in_=m2m[:].broadcast_to([128, M])).then_inc(dmC, 16)
            sync.dma_start(out=tgt[:], in_=tg[:].rearrange("(p f) -> p f", f=32)).then_inc(dmD, 16)
            sync.dma_start(out=wrow[:], in_=wm[:].broadcast_to([2, M])).then_inc(dmD, 16)
            sync.dma_start(out=mrow[0:1, :], in_=m1m[:]).then_inc(dmD, 16)
            sync.dma_start(out=mrow[1:2, :], in_=m2m[:]).then_inc(dmD, 16)
            # V row-1 moves
            sync.wait_ge(sa2, SA2B + 2)
            sync.dma_start(out=V[1:2, 0:M], in_=st_b2[1:2, :]).then_inc(dmV, 16)
            sync.wait_ge(sa2, SA2B + 4)
            sync.dma_start(out=V[1:2, M : 2 * M], in_=st_p2[1:2, :]).then_inc(dmV, 16)
            # outputs
            sync.wait_ge(sv, SVB + SV_IT)
            sync.dma_start(out=out[0:12], in_=Gsb[:]).then_inc(dmV, 16)
            sync.dma_start(out=out[12:28], in_=sc2[:]).then_inc(dmV, 16)

        @block.gpsimd
        def _(gp):
            gp.memset(ones_t[:], 1.0)

        @block.scalar
        def _(scalar):
            scalar.wait_ge(dmA, DMAB + 16)
            scalar.activation(em[:], lgtm[:], AF.Exp).then_inc(sa, 1)
            scalar.wait_ge(dmA, DMAB + 32)
            scalar.activation(e[:], lgt[:], AF.Exp).then_inc(sa, 1)
            scalar.wait_ge(svp, SVPB + 4)  # den
            scalar.activation(lnden[:], den[:], AF.Ln).then_inc(sa, 1)
            # b tiles: start as soon as ny1/ny2 + y rows are in
            scalar.wait_ge(svp, SVPB + 1)
            scalar.wait_ge(dmC, DMCB + 32)
            for K in range(nb // 2):
                s = K % DEPTH
                spw = SPBB + K - DEPTH + 1
                if spw > 0:
                    scalar.wait_ge(spb, spw)
                k0, k1 = 2 * K, 2 * K + 1
                scalar.activation(b1_[s][:, 0:M], y1row[:], AF.Abs, bias=ny1[:, k0 : k0 + 1]).then_inc(sa, 1)
                scalar.activation(b1_[s][:, M : 2 * M], y1row[:], AF.Abs, bias=ny1[:, k1 : k1 + 1]).then_inc(sa, 1)
                scalar.activation(b2_[s][:, 0:M], y2row[:], AF.Abs, bias=ny2[:, k0 : k0 + 1]).then_inc(sa, 1)
                scalar.activation(b2_[s][:, M : 2 * M], y2row[:], AF.Abs, bias=ny2[:, k1 : k1 + 1]).then_inc(sa, 1)
            # V assembly: partition-0 copies; row-1 halves moved by SP DMAs
            scalar.wait_ge(spb, SPBB + nb // 2)
            scalar.activation(V[0:1, 0:M], ps_b1[0:1, :], AF.Copy).then_inc(sa2, 1)
            scalar.activation(st_b2[:], ps_b2[:], AF.Copy).then_inc(sa2, 1)
            scalar.wait_ge(sp, SPB + nb // 2)
            scalar.activation(V[0:1, M : 2 * M], ps_p1[0:1, :], AF.Copy).then_inc(sa2, 1)
            scalar.activation(st_p2[:], ps_p2[:], AF.Copy).then_inc(sa2, 1)

        @block.vector
        def _(vector):
            # prep: negated masses, masked weights, bf16 weight pairs
            vector.wait_ge(dmB, DMBB + 48)
            vector.tensor_scalar(ny1[:], m1t[:], -1.0, None, AO.mult)
            vector.tensor_scalar(ny2[:], m2t[:], -1.0, None, AO.mult)
            vector.drain().then_inc(svp, 1)
            vector.tensor_scalar(msk1[:], m1t[:], 0.0, None, AO.is_gt)
            vector.tensor_scalar(msk2[:], m2t[:], 0.0, None, AO.is_gt)
            vector.tensor_tensor(w1[:], wt[:], msk1[:], AO.mult)
            vector.tensor_tensor(w2[:], wt[:], msk2[:], AO.mult)
            vector.drain()
            vector.tensor_copy(w12[:, 0:64:2], w1[:])
            vector.tensor_copy(w12[:, 1:64:2], w2[:])
            vector.drain().then_inc(svp, 1)
            # own-row softmax scores (gate the transpose/broadcast chain)
            vector.wait_ge(sa, SAB + 1)
            vector.tensor_reduce(denm[:], em[:].rearrange("p (f c) -> p f c", c=3), AX.X, AO.add)
            vector.drain()
            vector.reciprocal(recm[:], denm[:])
            vector.drain()
            vector.tensor_tensor(scm[:], em[:, 0:12:3], recm[:], AO.mult)
            vector.drain()
            vector.tensor_copy(scmbf[:], scm[:]).then_inc(svp, 1)
            # full-N scores
            vector.wait_ge(sa, SAB + 2)
            vector.tensor_reduce(den[:], e[:].rearrange("p (f c) -> p f c", c=3), AX.X, AO.add)
            vector.drain()
            vector.reciprocal(rec[:], den[:])
            vector.drain()
            vector.tensor_tensor(sc[:], e[:, 0:96:3], rec[:], AO.mult)
            vector.drain()
            vector.tensor_copy(scbf[:], sc[:])
            vector.drain()
            vector.tensor_copy(sc_r[:], scbf[:]).then_inc(svp, 1)
            # own-score broadcast: PSUM transpose -> SBUF -> PSUM bcast -> xrow
            vector.wait_ge(spx, SPXB + 1)
            vector.tensor_copy(T4sb[:], ps_t[:]).then_inc(svp, 1)
            vector.wait_ge(spx, SPXB + 2)
            vector.tensor_copy(xrow[:], ps_x[:])
            # ---- pairwise loop, with CE/G statistics interleaved into
            # the per-iteration slack (DVE is ~0.5us/group lighter than ACT) ----
            def stats(K):
                if K == 1:
                    vector.wait_ge(dmD, DMDB + 64)
                    vector.tensor_copy(tgtf[:], tgt[:])
                    vector.tensor_copy(m1ff[:], m1t[:])
                elif K == 2:
                    vector.tensor_copy(m2ff[:], m2t[:])
                    vector.tensor_tensor(sq[:], sc_r[:], sc_r[:], AO.mult)
                elif K == 3:
                    vector.scalar_tensor_tensor(lt[:], tgtf[:], 0.0, lgt[:, 0:96:3], AO.is_equal, AO.mult)
                    vector.scalar_tensor_tensor(pr[:], tgtf[:], 1.0, lgt[:, 1:96:3], AO.is_equal, AO.mult)
                elif K == 4:
                    vector.tensor_tensor(lt[:], lt[:], pr[:], AO.add)
                    vector.scalar_tensor_tensor(pr[:], tgtf[:], 2.0, lgt[:, 2:96:3], AO.is_equal, AO.mult)
                elif K == 5:
                    vector.tensor_tensor(lt[:], lt[:], pr[:], AO.add)
                    vector.tensor_scalar(mrow_w[:], mrow[:], 0.0, None, AO.is_gt)
                elif K == 6:
                    vector.tensor_tensor(w12row[:], wrow[:], mrow_w[:], AO.mult)
                elif K == 7:
                    vector.tensor_copy(w12row_bf[:], w12row[:])
                elif K == 8:
                    vector.wait_ge(sa, SAB + 3)  # lnden
                    vector.tensor_tensor(ce[:], lnden[:], lt[:], AO.subtract)
                elif K == 9:
                    vector.scalar_tensor_tensor(pr[:], wt[:], 0.0, wt[:], AO.is_ge, AO.mult, accum_out=G[:, 0:1])
                    vector.scalar_tensor_tensor(pr[:], wt[:], 1.0, msk1[:], AO.mult, AO.mult, accum_out=G[:, 1:2])
                elif K == 10:
                    vector.scalar_tensor_tensor(pr[:], wt[:], 1.0, msk2[:], AO.mult, AO.mult, accum_out=G[:, 2:3])
                    vector.scalar_tensor_tensor(pr[:], wt[:], 1.0, ce[:], AO.mult, AO.mult, accum_out=G[:, 3:4])
                elif K == 11:
                    vector.scalar_tensor_tensor(pr[:], w1[:], 1.0, sc_r[:], AO.mult, AO.mult, accum_out=G[:, 4:5])
                    vector.scalar_tensor_tensor(pr[:], w1[:], 1.0, sq[:], AO.mult, AO.mult, accum_out=G[:, 5:6])
                elif K == 12:
                    vector.scalar_tensor_tensor(pra[:], w1[:], 1.0, m1ff[:], AO.mult, AO.mult, accum_out=G[:, 6:7])
                    vector.scalar_tensor_tensor(pr[:], w2[:], 1.0, sc_r[:], AO.mult, AO.mult, accum_out=G[:, 8:9])
                elif K == 13:
                    vector.scalar_tensor_tensor(pr[:], pra[:], 1.0, m1ff[:], AO.mult, AO.mult, accum_out=G[:, 7:8])
                    vector.scalar_tensor_tensor(pr[:], w2[:], 1.0, sq[:], AO.mult, AO.mult, accum_out=G[:, 9:10])
                elif K == 14:
                    vector.scalar_tensor_tensor(pra[:], w2[:], 1.0, m2ff[:], AO.mult, AO.mult, accum_out=G[:, 10:11])
                elif K == 15:
                    vector.scalar_tensor_tensor(pr[:], pra[:], 1.0, m2ff[:], AO.mult, AO.mult, accum_out=G[:, 11:12])
                    vector.drain().then_inc(svp, 1)

            for K in range(nb // 2):
                s = K % DEPTH
                spw = SPB + K - DEPTH + 1 if K >= DEPTH else SPB + K - DEPTH
                if spw > 0:
                    vector.wait_ge(sp, spw)
                if nb == NB:
                    stats(K)
                k0, k1 = 2 * K, 2 * K + 1
                vector.tensor_scalar(d_[s][:, 0:M], xrow[:], sc_r[:, k0 : k0 + 1], None, AO.subtract)
                vector.tensor_scalar(d_[s][:, M : 2 * M], xrow[:], sc_r[:, k1 : k1 + 1], None, AO.subtract)
                vector.tensor_scalar(
                    a_[s][:].bitcast(U16), d_[s][:].bitcast(U16), 0x7FFF, None, AO.bitwise_and
                ).then_inc(sv, 1)
                vector.wait_ge(sa, SAB + sa_val(K, 2))
                vector.tensor_tensor(ab1_[s][:], a_[s][:], b1_[s][:], AO.mult)
                vector.wait_ge(sa, SAB + sa_val(K, 4))
                vector.tensor_tensor(ab2_[s][:], a_[s][:], b2_[s][:], AO.mult).then_inc(sv, 1)
            if nb != NB:
                # timing variants: emit all stats after the loop
                for K in range(16):
                    stats(K)
            # ---- phase 2: fused dot products via stt-accum ----
            vector.wait_ge(sp, SPB + nb // 2)
            vector.tensor_copy(arsb[:], ps_a[:])
            vector.drain()
            vector.scalar_tensor_tensor(t_[:], arsb[:], 1.0, w12row_bf[:], AO.mult, AO.mult, accum_out=sc2[:, 0:1])   # g_a
            vector.scalar_tensor_tensor(t3_[:], t_[:], 1.0, arsb[:], AO.mult, AO.mult, accum_out=sc2[:, 1:2])      # T1aa
            vector.wait_ge(sa2, SA2B + 1)
            vector.wait_ge(dmV, DMVB + 16)
            vector.scalar_tensor_tensor(t3_[:], t_[:], 1.0, V[:, 0:M], AO.mult, AO.mult, accum_out=sc2[:, 4:5])    # T1ab
            vector.scalar_tensor_tensor(t2_[:], V[:, 0:M], 1.0, w12row_bf[:], AO.mult, AO.mult, accum_out=sc2[:, 2:3])  # g_b
            vector.scalar_tensor_tensor(t3_[:], t2_[:], 1.0, V[:, 0:M], AO.mult, AO.mult, accum_out=sc2[:, 3:4])   # T1bb
            vector.wait_ge(sa2, SA2B + 3)
            vector.wait_ge(dmV, DMVB + 32)
            vector.scalar_tensor_tensor(t3_[:], V[:, M : 2 * M], 1.0, w12row_bf[:], AO.mult, AO.mult, accum_out=sc2[:, 5:6])  # S_ab
            vector.wait_ge(sp, SPB + nb // 2 + 1)
            vector.tensor_copy(Gsb[:], ps_g[:]).then_inc(sv, 200)

        @block.tensor
        def _(tensor):
            def bpart(J):
                s = J % DEPTH
                st = J == 0
                last = J == nb // 2 - 1
                k0, k1 = 2 * J, 2 * J + 1
                lw0 = w12[:, 2 * k0 : 2 * k0 + 2]
                lw1 = w12[:, 2 * k1 : 2 * k1 + 2]
                tensor.wait_ge(sa, SAB + sa_val(J, 4))
                tensor.matmul(ps_b1[:], lw0, b1_[s][:, 0:M], start=st, stop=False)
                tensor.matmul(ps_b2[:], lw0, b2_[s][:, 0:M], start=st, stop=False)
                tensor.matmul(ps_b1[:], lw1, b1_[s][:, M : 2 * M], start=False, stop=last)
                tensor.matmul(ps_b2[:], lw1, b2_[s][:, M : 2 * M], start=False, stop=last).then_inc(spb, 1)

            def abpart(K):
                s = K % DEPTH
                st = K == 0
                last = K == nb // 2 - 1
                k0, k1 = 2 * K, 2 * K + 1
                lw0 = w12[:, 2 * k0 : 2 * k0 + 2]
                lw1 = w12[:, 2 * k1 : 2 * k1 + 2]
                tensor.wait_ge(sv, SVB + 2 * K + 2)
                tensor.matmul(ps_a[:], lw0, a_[s][:, 0:M], start=st, stop=False)
                tensor.matmul(ps_p1[:], lw0, ab1_[s][:, 0:M], start=st, stop=False)
                tensor.matmul(ps_p2[:], lw0, ab2_[s][:, 0:M], start=st, stop=False)
                tensor.matmul(ps_a[:], lw1, a_[s][:, M : 2 * M], start=False, stop=last)
                tensor.matmul(ps_p1[:], lw1, ab1_[s][:, M : 2 * M], start=False, stop=last)
                tensor.matmul(ps_p2[:], lw1, ab2_[s][:, M : 2 * M], start=False, stop=last).then_inc(sp, 1)

            # own-score transpose + broadcast (idn/W4 are constant inputs)
            tensor.wait_ge(dmE, 32)
            tensor.wait_ge(svp, SVPB + 3)
            tensor.transpose(ps_t[:], scmbf[:], idn[:]).then_inc(spx, 1)
            tensor.wait_ge(svp, SVPB + 5)
            for c in range(4):
                mm = tensor.matmul(ps_x[:, 128 * c : 128 * (c + 1)], W4[:, 128 * c : 128 * (c + 1)], T4sb[:], start=True, stop=True)
            mm.then_inc(spx, 1)
            tensor.wait_ge(svp, SVPB + 2)  # w12 ready
            bpart(0)
            for K in range(nb // 2):
                if K + 1 < nb // 2:
                    bpart(K + 1)
                abpart(K)
            tensor.wait_ge(svp, SVPB + 6)  # G ready
            tensor.matmul(ps_g[:], ones_t[:], G[:], start=True, stop=True).then_inc(sp, 1)

    return nc, es


_NC_CACHE = {}


def _get_runner(reps=1, nb=NB):
    """Build (once) and cache the program + AOT-jitted single-dispatch callable."""
    key = ("runner", reps, nb)
    if key in _NC_CACHE:
        return _NC_CACHE[key]

    import jax
    from jax.sharding import Mesh, PartitionSpec
    from jax.experimental.shard_map import shard_map
    import concourse.bass2jax as b2j

    nc, _es = _build_program(reps, nb)
    b2j.install_neuronx_cc_hook()

    partition_name = nc.partition_id_tensor.name if nc.partition_id_tensor else None
    in_names, out_names, out_avals, out_shapes = [], [], [], []
    for alloc in nc.m.functions[0].allocations:
        if not isinstance(alloc, mybir.MemoryLocationSet):
            continue
        name = alloc.memorylocations[0].name
        if alloc.kind == "ExternalInput":
            if name != partition_name:
                in_names.append(name)
        elif alloc.kind == "ExternalOutput":
            out_names.append(name)
            shape = tuple(alloc.tensor_shape)
            dtype = mybir.dt.np(alloc.dtype)
            out_avals.append(jax.core.ShapedArray(shape, dtype))
            out_shapes.append((shape, dtype))
    n_params = len(in_names)
    n_outs = len(out_names)
    in_names_all = list(in_names) + list(out_names)
    if partition_name is not None:
        in_names_all.append(partition_name)

    def _body(*args):
        operands = list(args)
        if partition_name is not None:
            operands.append(b2j.partition_id_tensor())
        outs = b2j._bass_exec_p.bind(
            *operands,
            out_avals=tuple(out_avals),
            in_names=tuple(in_names_all),
            out_names=tuple(out_names),
            lowering_input_output_aliases=(),
            sim_require_finite=True,
            sim_require_nnan=True,
            nc=nc,
        )
        return tuple(outs)

    devices = jax.devices()[:NCORES]
    mesh = Mesh(np.asarray(devices), ("core",))
    in_specs = (PartitionSpec("core"),) * (n_params + n_outs)
    out_specs = (PartitionSpec("core"),) * n_outs
    jitted = jax.jit(
        shard_map(_body, mesh=mesh, in_specs=in_specs, out_specs=out_specs, check_rep=False),
        keep_unused=True,
    )

    def run(in_maps):
        concat_in = [
            np.concatenate([np.asarray(in_maps[c][name]) for c in range(NCORES)], axis=0)
            for name in in_names
        ]
        zeros = [np.zeros((NCORES * s[0], *s[1:]), d) for s, d in out_shapes]
        outs = jitted(*concat_in, *zeros)
        jax.block_until_ready(outs)
        return [
            {
                name: np.asarray(outs[i]).reshape(NCORES, *out_shapes[i][0])[c]
                for i, name in enumerate(out_names)
            }
            for c in range(NCORES)
        ]

    _NC_CACHE[key] = run
    return run


_IDN = np.eye(128, dtype=ml_dtypes.bfloat16)
_W4 = np.zeros((4, 512), dtype=ml_dtypes.bfloat16)
for _c in range(4):
    _W4[_c, 128 * _c : 128 * (_c + 1)] = 1


def _make_in_maps(logits, target_i, weight, m1b, m2b):
    in_maps = []
    for c in range(NCORES):
        sl = slice(c * M, (c + 1) * M)
        in_maps.append(
            {
                "idn": _IDN,
                "w4": _W4,
                "lg": logits,
                "lgm": np.ascontiguousarray(logits[sl]),
                "tg": target_i,
                "wf": weight,
                "wm": weight[sl].reshape(1, M),
                "m1f": m1b,
                "m2f": m2b,
                "m1m": m1b[sl].reshape(1, M),
                "m2m": m2b[sl].reshape(1, M),
            }
        )
    return in_maps


def kernel(logits, target, weight, mass1, mass2):
    logits = np.asarray(logits, dtype=np.float32)
    target_i = np.asarray(target).astype(np.int32)
    weight = np.asarray(weight, dtype=np.float32)
    mass1 = np.asarray(mass1, dtype=np.float32)
    mass2 = np.asarray(mass2, dtype=np.float32)
    m1b = mass1.astype(ml_dtypes.bfloat16)
    m2b = mass2.astype(ml_dtypes.bfloat16)

    run = _get_runner(reps=1)
    res = run(_make_in_maps(logits, target_i, weight, m1b, m2b))
    outs = [r["out"] for r in res]
    return _combine(outs)


def _combine(outs):
    G = np.asarray(outs[0][0:12], dtype=np.float64)
    Sw, S1, S2, CEs = G[0], G[1], G[2], G[3]
    m1, q1, my1, qy1 = G[4], G[5], G[6], G[7]
    m2, q2, my2, qy2 = G[8], G[9], G[10], G[11]
    # per-core partials: sc2 [2,8] flattened at out[12:28]
    P = np.zeros((2, 8), dtype=np.float64)
    for o in outs:
        P += o[12:28].reshape(2, 8).astype(np.float64)
    ce_mean = CEs / max(Sw, EPS_W)

    def disco(row, Sr, m, q, my, qy):
        g_a, T1aa, g_b, T1bb, T1ab, S_ab = P[row, 0:6]
        s = 1.0 / max(Sr, EPS_W)
        dcov = s * s * S_ab - 2.0 * s**3 * T1ab + s**4 * g_a * g_b
        dvx = 2.0 * (s * q - (s * m) ** 2) - 2.0 * s**3 * T1aa + (s * s * g_a) ** 2
        dvy = 2.0 * (s * qy - (s * my) ** 2) - 2.0 * s**3 * T1bb + (s * s * g_b) ** 2
        ok = (dvx > EPS_VAR) and (dvy > EPS_VAR)
        if not ok:
            return 0.0
        return np.sqrt(np.abs(dcov) / np.sqrt(dvx * dvy))

    d1 = disco(0, S1, m1, q1, my1, qy1)
    d2 = disco(1, S2, m2, q2, my2, qy2)
    return np.float32(ce_mean + DISCO_LAMBDA * (d1 + d2))
